# revision 8
# baseline (speedup 1.0000x reference)
# Trainium2 Bass kernel for nn_Decoder (RNN decoder):
#   xp = x @ W_ih^T + b_ih + b_hh            (GEMM1, fp32r)
#   h_t = tanh(xp_t + h_{t-1} @ W_hh^T)      (512-step recurrence, bf16 matmul)
#   y  = hs @ W_ff^T + b_ff                  (GEMM2, bf16)
#
# Sharding: data-parallel over batch, 8 batch rows per core, weights replicated.
# Layouts are hidden-on-partitions so the sequential recurrence needs no
# transposes: h is stored [hid(4x128 part-tiles), batch(8)] and the whole
# history hs doubles as GEMM2's input.

import numpy as np

B, S, I, H, O = 64, 512, 256, 512, 256
NCORES = 8
BL = B // NCORES  # 8 batch rows per core
P = 128
KI, KH, KO = I // P, H // P, O // P  # 2, 4, 2
CH = 512                             # free-dim chunk for the big GEMMs
_builder_cache = {}


def build_nc(seq_len=S):
    """Build the (single-core SPMD) Bass program for sequence length seq_len."""
    import concourse.bass as bass
    import concourse.mybir as mybir
    import concourse.tile as tile
    from concourse import bacc

    f32 = mybir.dt.float32
    f32r = mybir.dt.float32r
    bf16 = mybir.dt.bfloat16
    AF = mybir.ActivationFunctionType

    s = seq_len
    F = s * BL               # free length of (t, b) axis
    nch = max(1, F // CH)    # chunks for GEMM1/GEMM2
    ch = F // nch

    nc = bacc.Bacc("TRN2")

    xt = nc.dram_tensor("xt", [I, F], f32, kind="ExternalInput")       # x^T  (in, t*BL+b)
    h0t = nc.dram_tensor("h0t", [H, BL], f32, kind="ExternalInput")    # h0^T (hid, b)
    wih = nc.dram_tensor("wih", [I, H], f32, kind="ExternalInput")     # W_ih^T
    whh = nc.dram_tensor("whh", [H, H], f32, kind="ExternalInput")     # W_hh^T
    wff = nc.dram_tensor("wff", [H, O], f32, kind="ExternalInput")     # W_ff^T
    bcb = nc.dram_tensor("bcb", [P, KH], f32, kind="ExternalInput")    # b_ih+b_hh, [128, 4]
    bfb = nc.dram_tensor("bfb", [P, KO], f32, kind="ExternalInput")    # b_ff, [128, 2]
    y = nc.dram_tensor("y", [KO, P, F], f32, kind="ExternalOutput")

    with tile.TileContext(nc) as tc:
        with (
            tc.tile_pool(name="const", bufs=1) as cp,
            tc.tile_pool(name="big", bufs=1) as bp,
        ):
            wih_sb = cp.tile([P, KI, H], bf16)
            whh_sb = cp.tile([P, KH, H], bf16)
            wff_sb = cp.tile([P, KH, O], bf16)
            bcb_sb = cp.tile([P, KH], f32)
            bfb_sb = cp.tile([P, KO], f32)

            xt_sb = bp.tile([P, KI, F], bf16)
            xp_sb = bp.tile([P, KH, F], f32)
            hs_sb = bp.tile([P, KH, (s + 1) * BL], bf16)
            out_sb = bp.tile([P, KO, F], f32)

            # ---- constant loads (gpsimd = SWDGE does the fp32->bf16 casts) ----
            nc.gpsimd.dma_start(wih_sb[:], wih[:].rearrange("(k p) h -> p k h", p=P))
            nc.gpsimd.dma_start(whh_sb[:], whh[:].rearrange("(k p) h -> p k h", p=P))
            nc.gpsimd.dma_start(wff_sb[:], wff[:].rearrange("(k p) o -> p k o", p=P))
            nc.sync.dma_start(bcb_sb[:], bcb[:])
            nc.sync.dma_start(bfb_sb[:], bfb[:])
            nc.gpsimd.dma_start(
                hs_sb[:, :, 0:BL], h0t[:].rearrange("(k p) b -> p k b", p=P)
            )

            xt_r = xt[:].rearrange("(k p) f -> p k f", p=P)

            # ---- GEMM1: xp[hid, (t,b)] = W_ih @ x^T + (b_ih + b_hh) ----
            with tc.tile_pool(name="g1ps", bufs=3, space=bass.MemorySpace.PSUM) as g1p:
                for j in range(nch):
                    sl = slice(j * ch, (j + 1) * ch)
                    nc.gpsimd.dma_start(xt_sb[:, :, sl], xt_r[:, :, sl])
                    for m in range(KH):
                        ps = g1p.tile([P, ch], f32)
                        for k in range(KI):
                            nc.tensor.matmul(
                                ps[:],
                                wih_sb[:, k, m * P : (m + 1) * P],
                                xt_sb[:, k, sl],
                                start=(k == 0),
                                stop=(k == KI - 1),
                            )
                        dst = xp_sb[:, m, sl]
                        nc.scalar.activation(
                            dst, ps[:], AF.Identity, bias=bcb_sb[:, m : m + 1]
                        )

            # ---- recurrence ----
            # z tile: [128, 4, 512] fp32 = 4 PSUM banks, one bank per output
            # hid-tile m (cols 0:BL used).  bufs=2 double-buffers across steps.
            with tc.tile_pool(name="zps", bufs=2, space=bass.MemorySpace.PSUM) as zp:
                for t in range(s):
                    z = zp.tile([P, KH, 512], f32)
                    hprev = slice(t * BL, (t + 1) * BL)
                    hnext = slice((t + 1) * BL, (t + 2) * BL)
                    for m in range(KH):
                        for k in range(KH):
                            nc.tensor.matmul(
                                z[:, m, 0:BL],
                                whh_sb[:, k, m * P : (m + 1) * P],
                                hs_sb[:, k, hprev],
                                start=(k == 0),
                                stop=(k == KH - 1),
                            )
                        if m == 0:
                            # tile-A (hid 0:128) finishes first; its tanh
                            # unblocks next step's k=0 matmuls early.
                            nc.vector.tensor_add(
                                z[:, 0:1, 0:BL], z[:, 0:1, 0:BL], xp_sb[:, 0:1, hprev]
                            )
                            nc.scalar.activation(
                                hs_sb[:, 0:1, hnext], z[:, 0:1, 0:BL], AF.Tanh
                            )
                    nc.vector.tensor_add(
                        z[:, 1:KH, 0:BL], z[:, 1:KH, 0:BL], xp_sb[:, 1:KH, hprev]
                    )
                    nc.scalar.activation(
                        hs_sb[:, 1:KH, hnext], z[:, 1:KH, 0:BL], AF.Tanh
                    )

            # ---- GEMM2: y[(o), (t,b)] = W_ff @ hs + b_ff ----
            y_r = y[:].rearrange("o p f -> p o f")
            with tc.tile_pool(name="g2ps", bufs=3, space=bass.MemorySpace.PSUM) as g2p:
                for j in range(nch):
                    sl = slice(j * ch, (j + 1) * ch)
                    hsl = slice(BL + j * ch, BL + (j + 1) * ch)
                    for ot in range(KO):
                        ps = g2p.tile([P, ch], f32)
                        for k in range(KH):
                            nc.tensor.matmul(
                                ps[:],
                                wff_sb[:, k, ot * P : (ot + 1) * P],
                                hs_sb[:, k, hsl],
                                start=(k == 0),
                                stop=(k == KH - 1),
                            )
                        dst = out_sb[:, ot, sl]
                        nc.scalar.activation(
                            dst, ps[:], AF.Identity, bias=bfb_sb[:, ot : ot + 1]
                        )
                    nc.sync.dma_start(y_r[:, :, sl], out_sb[:, :, sl])

    return nc


def make_in_maps(x, h0, W_ih, W_hh, b_ih, b_hh, W_ff, b_ff, seq_len=S):
    """Host-side sharding + layout prep: per-core input dicts."""
    x = np.asarray(x, np.float32)
    h0 = np.asarray(h0, np.float32)
    wih = np.ascontiguousarray(np.asarray(W_ih, np.float32).T)  # [I, H]
    whh = np.ascontiguousarray(np.asarray(W_hh, np.float32).T)  # [H, H]
    wff = np.ascontiguousarray(np.asarray(W_ff, np.float32).T)  # [H, O]
    bc = np.asarray(b_ih, np.float32) + np.asarray(b_hh, np.float32)
    bcb = np.ascontiguousarray(bc.reshape(KH, P).T)             # [128, KH]
    bfb = np.ascontiguousarray(np.asarray(b_ff, np.float32).reshape(KO, P).T)

    in_maps = []
    for c in range(NCORES):
        xs = x[c * BL : (c + 1) * BL, :seq_len]                 # [BL, s, I]
        xt = np.ascontiguousarray(xs.transpose(2, 1, 0)).reshape(I, seq_len * BL)
        h0t = np.ascontiguousarray(h0[c * BL : (c + 1) * BL].T)  # [H, BL]
        in_maps.append(
            {
                "xt": xt,
                "h0t": h0t,
                "wih": wih,
                "whh": whh,
                "wff": wff,
                "bcb": bcb,
                "bfb": bfb,
            }
        )
    return in_maps


def assemble_output(results, seq_len=S):
    """Per-core y [KO, 128, s*BL] -> full [B, s, O]."""
    outs = []
    for r in results:
        yc = np.asarray(r["y"]).reshape(O, seq_len, BL).transpose(2, 1, 0)
        outs.append(yc)
    return np.ascontiguousarray(np.concatenate(outs, axis=0))


def _get_finalized_nc(seq_len=S):
    key = ("nc", seq_len)
    if key not in _builder_cache:
        nc = build_nc(seq_len)
        nc.finalize()
        _builder_cache[key] = nc
    return _builder_cache[key]


def run_on_cores(inputs, seq_len=S, **kwargs):
    from concourse.bass_utils import run_bass_kernel_spmd

    nc = _get_finalized_nc(seq_len)
    in_maps = make_in_maps(**inputs, seq_len=seq_len)
    res = run_bass_kernel_spmd(nc, in_maps, core_ids=list(range(NCORES)), **kwargs)
    return res


def kernel(**inputs) -> np.ndarray:
    res = run_on_cores(inputs)
    return assemble_output(res.results)


# revision 10
# speedup vs baseline: 1.2688x; 1.2688x over previous
# Trainium2 Bass kernel for nn_Decoder (RNN decoder):
#   xp = x @ W_ih^T + b_ih + b_hh            (GEMM1, bf16)
#   h_t = tanh(xp_t + h_{t-1} @ W_hh^T)      (512-step recurrence, bf16 matmul)
#   y  = hs @ W_ff^T + b_ff                  (GEMM2, bf16)
#
# Sharding: data-parallel over batch, 8 batch rows per core, weights replicated.
# Layouts are hidden-on-partitions so the sequential recurrence needs no
# transposes: h is stored [hid(4x128 part-tiles), batch(8)].
#
# Recurrence structure per step (critical path):
#   - per output hid-tile m (its own PSUM bank): identity-matmul injects xp_t
#     (start=True, dep-free), then 4 W_hh^T k-tile matmuls accumulate W@h.
#   - tanh on ACT, split 1+3 so tile 0's tanh lands early and unblocks the
#     next step's k=0 matmuls.
#   - h history is kept in two parity-alternating tensors (hs0/hs1) so the
#     dependency tracker sees minimal cross-step deps; GEMM2 consumes both.

import numpy as np

B, S, I, H, O = 64, 512, 256, 512, 256
NCORES = 8
BL = B // NCORES  # 8 batch rows per core
P = 128
KI, KH, KO = I // P, H // P, O // P  # 2, 4, 2
CH = 512                             # free-dim chunk for the big GEMMs
_builder_cache = {}


def build_nc(seq_len=S):
    """Build the (single-core SPMD) Bass program for sequence length seq_len."""
    import concourse.bass as bass
    import concourse.mybir as mybir
    import concourse.tile as tile
    from concourse import bacc

    f32 = mybir.dt.float32
    bf16 = mybir.dt.bfloat16
    AF = mybir.ActivationFunctionType

    s = seq_len
    assert s % 2 == 0
    F = s * BL               # free length of (t, b) axis
    nch = max(1, F // CH)    # chunks for GEMM1
    ch = F // nch
    F2 = (F // 2)            # per-parity free length for GEMM2
    nch2 = max(1, F2 // CH)
    ch2 = F2 // nch2

    nc = bacc.Bacc("TRN2")

    xt = nc.dram_tensor("xt", [I, F], f32, kind="ExternalInput")       # x^T  (in, t*BL+b)
    h0t = nc.dram_tensor("h0t", [H, BL], f32, kind="ExternalInput")    # h0^T (hid, b)
    wih = nc.dram_tensor("wih", [I, H], f32, kind="ExternalInput")     # W_ih^T
    whh = nc.dram_tensor("whh", [H, H], f32, kind="ExternalInput")     # W_hh^T
    wff = nc.dram_tensor("wff", [H, O], f32, kind="ExternalInput")     # W_ff^T
    bcb = nc.dram_tensor("bcb", [P, KH], f32, kind="ExternalInput")    # b_ih+b_hh, [128, 4]
    bfb = nc.dram_tensor("bfb", [P, KO], f32, kind="ExternalInput")    # b_ff, [128, 2]
    eye = nc.dram_tensor("eye", [P, P], f32, kind="ExternalInput")
    # y[ot, p, par, s*BL + b]:  par=0 -> t = 2s+1 (hs0 slot s+1), par=1 -> t = 2s (hs1 slot s)
    y = nc.dram_tensor("y", [KO, P, 2, F2], f32, kind="ExternalOutput")

    with tile.TileContext(nc) as tc:
        with (
            tc.tile_pool(name="const", bufs=1) as cp,
            tc.tile_pool(name="big", bufs=1) as bp,
        ):
            wih_sb = cp.tile([P, KI, H], bf16)
            whh_sb = cp.tile([P, KH, H], bf16)
            wff_sb = cp.tile([P, KH, O], bf16)
            bcb_sb = cp.tile([P, KH], f32)
            bfb_sb = cp.tile([P, KO], f32)
            eye_sb = cp.tile([P, P], bf16)

            xt_sb = bp.tile([P, KI, F], bf16)
            xp_sb = bp.tile([P, KH, F], bf16)
            # h_i (i = t+1, 0..s) lives in hs[i % 2] at col-slot (i // 2) * BL
            hs0 = bp.tile([P, KH, (s // 2 + 1) * BL], bf16)
            hs1 = bp.tile([P, KH, (s // 2) * BL], bf16)
            hsx = [hs0, hs1]
            out_sb = bp.tile([P, KO, 2, F2], f32)

            # ---- constant loads (gpsimd = SWDGE does the fp32->bf16 casts) ----
            nc.gpsimd.dma_start(wih_sb[:], wih[:].rearrange("(k p) h -> p k h", p=P))
            nc.gpsimd.dma_start(whh_sb[:], whh[:].rearrange("(k p) h -> p k h", p=P))
            nc.gpsimd.dma_start(wff_sb[:], wff[:].rearrange("(k p) o -> p k o", p=P))
            nc.sync.dma_start(bcb_sb[:], bcb[:])
            nc.sync.dma_start(bfb_sb[:], bfb[:])
            nc.gpsimd.dma_start(eye_sb[:], eye[:])
            nc.gpsimd.dma_start(
                hs0[:, :, 0:BL], h0t[:].rearrange("(k p) b -> p k b", p=P)
            )

            xt_r = xt[:].rearrange("(k p) f -> p k f", p=P)

            # ---- GEMM1: xp[hid, (t,b)] = W_ih @ x^T + (b_ih + b_hh) ----
            with tc.tile_pool(name="g1ps", bufs=3, space=bass.MemorySpace.PSUM) as g1p:
                for j in range(nch):
                    sl = slice(j * ch, (j + 1) * ch)
                    nc.gpsimd.dma_start(xt_sb[:, :, sl], xt_r[:, :, sl])
                    for m in range(KH):
                        ps = g1p.tile([P, ch], f32)
                        for k in range(KI):
                            nc.tensor.matmul(
                                ps[:],
                                wih_sb[:, k, m * P : (m + 1) * P],
                                xt_sb[:, k, sl],
                                start=(k == 0),
                                stop=(k == KI - 1),
                            )
                        nc.scalar.activation(
                            xp_sb[:, m, sl], ps[:], AF.Identity,
                            bias=bcb_sb[:, m : m + 1],
                        )

            # ---- recurrence ----
            # z tile: [128, 4, 512] fp32 = 4 PSUM banks, one bank per output
            # hid-tile m.  bufs=2 double-buffers across steps.
            with tc.tile_pool(name="zps", bufs=2, space=bass.MemorySpace.PSUM) as zp:
                for t in range(s):
                    z = zp.tile([P, KH, 512], f32)
                    rbuf = hsx[t % 2]
                    wbuf = hsx[(t + 1) % 2]
                    rof = (t // 2) * BL
                    wof = ((t + 1) // 2) * BL
                    for m in range(KH):
                        nc.tensor.matmul(
                            z[:, m, 0:BL],
                            eye_sb[:],
                            xp_sb[:, m, t * BL : (t + 1) * BL],
                            start=True,
                            stop=False,
                        )
                        for k in range(KH):
                            nc.tensor.matmul(
                                z[:, m, 0:BL],
                                whh_sb[:, k, m * P : (m + 1) * P],
                                rbuf[:, k, rof : rof + BL],
                                start=False,
                                stop=(k == KH - 1),
                            )
                        if m == 0:
                            nc.scalar.activation(
                                wbuf[:, 0:1, wof : wof + BL],
                                z[:, 0:1, 0:BL],
                                AF.Tanh,
                            )
                    nc.scalar.activation(
                        wbuf[:, 1:KH, wof : wof + BL],
                        z[:, 1:KH, 0:BL],
                        AF.Tanh,
                    )

            # ---- GEMM2: y = W_ff @ hs + b_ff, per h-parity buffer ----
            # par=0 reads hs0 slots 1.. (t = 2s+1); par=1 reads hs1 slots 0.. (t = 2s)
            y_r = y[:].rearrange("o p q f -> p o q f")
            with tc.tile_pool(name="g2ps", bufs=3, space=bass.MemorySpace.PSUM) as g2p:
                for par in range(2):
                    base = BL if par == 0 else 0
                    for j in range(nch2):
                        sl = slice(j * ch2, (j + 1) * ch2)
                        hsl = slice(base + j * ch2, base + (j + 1) * ch2)
                        for ot in range(KO):
                            ps = g2p.tile([P, ch2], f32)
                            for k in range(KH):
                                nc.tensor.matmul(
                                    ps[:],
                                    wff_sb[:, k, ot * P : (ot + 1) * P],
                                    (hs0 if par == 0 else hs1)[:, k, hsl],
                                    start=(k == 0),
                                    stop=(k == KH - 1),
                                )
                            nc.scalar.activation(
                                out_sb[:, ot, par, sl], ps[:], AF.Identity,
                                bias=bfb_sb[:, ot : ot + 1],
                            )
                        nc.sync.dma_start(y_r[:, :, par, sl], out_sb[:, :, par, sl])

    return nc


def make_in_maps(x, h0, W_ih, W_hh, b_ih, b_hh, W_ff, b_ff, seq_len=S):
    """Host-side sharding + layout prep: per-core input dicts."""
    x = np.asarray(x, np.float32)
    h0 = np.asarray(h0, np.float32)
    wih = np.ascontiguousarray(np.asarray(W_ih, np.float32).T)  # [I, H]
    whh = np.ascontiguousarray(np.asarray(W_hh, np.float32).T)  # [H, H]
    wff = np.ascontiguousarray(np.asarray(W_ff, np.float32).T)  # [H, O]
    bc = np.asarray(b_ih, np.float32) + np.asarray(b_hh, np.float32)
    bcb = np.ascontiguousarray(bc.reshape(KH, P).T)             # [128, KH]
    bfb = np.ascontiguousarray(np.asarray(b_ff, np.float32).reshape(KO, P).T)
    eye = np.eye(P, dtype=np.float32)

    in_maps = []
    for c in range(NCORES):
        xs = x[c * BL : (c + 1) * BL, :seq_len]                 # [BL, s, I]
        xt = np.ascontiguousarray(xs.transpose(2, 1, 0)).reshape(I, seq_len * BL)
        h0t = np.ascontiguousarray(h0[c * BL : (c + 1) * BL].T)  # [H, BL]
        in_maps.append(
            {
                "xt": xt,
                "h0t": h0t,
                "wih": wih,
                "whh": whh,
                "wff": wff,
                "bcb": bcb,
                "bfb": bfb,
                "eye": eye,
            }
        )
    return in_maps


def assemble_output(results, seq_len=S):
    """Per-core y [KO, 128, 2, (s/2)*BL] -> full [B, s, O]."""
    s = seq_len
    outs = []
    for r in results:
        yc = np.asarray(r["y"]).reshape(O, 2, s // 2, BL)
        full = np.empty((O, s, BL), np.float32)
        full[:, 1::2, :] = yc[:, 0]   # par=0: t = 2q+1
        full[:, 0::2, :] = yc[:, 1]   # par=1: t = 2q
        outs.append(full.transpose(2, 1, 0))
    return np.ascontiguousarray(np.concatenate(outs, axis=0))


def _get_finalized_nc(seq_len=S):
    key = ("nc", seq_len)
    if key not in _builder_cache:
        nc = build_nc(seq_len)
        nc.finalize()
        _builder_cache[key] = nc
    return _builder_cache[key]


def run_on_cores(inputs, seq_len=S, **kwargs):
    from concourse.bass_utils import run_bass_kernel_spmd

    nc = _get_finalized_nc(seq_len)
    in_maps = make_in_maps(**inputs, seq_len=seq_len)
    res = run_bass_kernel_spmd(nc, in_maps, core_ids=list(range(NCORES)), **kwargs)
    return res


def kernel(**inputs) -> np.ndarray:
    res = run_on_cores(inputs)
    return assemble_output(res.results)


# revision 11
# speedup vs baseline: 1.8313x; 1.4434x over previous
# Trainium2 Bass kernel for nn_Decoder (RNN decoder):
#   xp = x @ W_ih^T + b_ih + b_hh            (GEMM1, bf16)
#   h_t = tanh(xp_t + h_{t-1} @ W_hh^T)      (512-step recurrence, bf16 matmul)
#   y  = hs @ W_ff^T + b_ff                  (GEMM2, bf16)
#
# Sharding: data-parallel over batch, 8 batch rows per core, weights replicated.
# Layouts are hidden-on-partitions so the sequential recurrence needs no
# transposes: h is stored [hid(4x128 part-tiles), batch(8)].
#
# Recurrence structure per step (critical path):
#   - output hid-tile m=0 accumulates in its own PSUM tensor z0 (1 bank),
#     tiles m=1..3 in zB (3 banks) — separate tensors so the dependency
#     tracker never serializes PE writes against the other half's tanh read.
#   - per bank: identity-matmul injects xp_t (start=True, dep-free), then 4
#     W_hh^T k-tile matmuls accumulate W@h.
#   - tanh split 1+3 on ACT: tile 0's tanh (actA) lands early and unblocks
#     the next step's k=0 matmuls while the other banks still accumulate.
#   - h history is split across 4 tensors by (step parity) x (A/B half) for
#     minimal tracker deps; GEMM2 consumes all four.

import numpy as np

B, S, I, H, O = 64, 512, 256, 512, 256
NCORES = 8
BL = B // NCORES  # 8 batch rows per core
P = 128
KI, KH, KO = I // P, H // P, O // P  # 2, 4, 2
CH = 512                             # free-dim chunk for the big GEMMs
_builder_cache = {}


def build_nc(seq_len=S):
    """Build the (single-core SPMD) Bass program for sequence length seq_len."""
    import concourse.bass as bass
    import concourse.mybir as mybir
    import concourse.tile as tile
    from concourse import bacc

    f32 = mybir.dt.float32
    bf16 = mybir.dt.bfloat16
    AF = mybir.ActivationFunctionType

    s = seq_len
    assert s % 2 == 0
    F = s * BL               # free length of (t, b) axis
    nch = max(1, F // CH)    # chunks for GEMM1
    ch = F // nch
    F2 = (F // 2)            # per-parity free length for GEMM2
    nch2 = max(1, F2 // CH)
    ch2 = F2 // nch2

    nc = bacc.Bacc("TRN2")

    xt = nc.dram_tensor("xt", [I, F], f32, kind="ExternalInput")       # x^T  (in, t*BL+b)
    h0t = nc.dram_tensor("h0t", [H, BL], f32, kind="ExternalInput")    # h0^T (hid, b)
    wih = nc.dram_tensor("wih", [I, H], f32, kind="ExternalInput")     # W_ih^T
    whh = nc.dram_tensor("whh", [H, H], f32, kind="ExternalInput")     # W_hh^T
    wff = nc.dram_tensor("wff", [H, O], f32, kind="ExternalInput")     # W_ff^T
    bcb = nc.dram_tensor("bcb", [P, KH], f32, kind="ExternalInput")    # b_ih+b_hh, [128, 4]
    bfb = nc.dram_tensor("bfb", [P, KO], f32, kind="ExternalInput")    # b_ff, [128, 2]
    eye = nc.dram_tensor("eye", [P, P], f32, kind="ExternalInput")
    # y[ot, p, par, q*BL + b]:  par=0 -> t = 2q+1, par=1 -> t = 2q
    y = nc.dram_tensor("y", [KO, P, 2, F2], f32, kind="ExternalOutput")

    with tile.TileContext(nc) as tc:
        with (
            tc.tile_pool(name="const", bufs=1) as cp,
            tc.tile_pool(name="big", bufs=1) as bp,
        ):
            wih_sb = cp.tile([P, KI, H], bf16)
            whh_sb = cp.tile([P, KH, H], bf16)
            wff_sb = cp.tile([P, KH, O], bf16)
            bcb_sb = cp.tile([P, KH], f32)
            bfb_sb = cp.tile([P, KO], f32)
            eye_sb = cp.tile([P, P], bf16)

            xt_sb = bp.tile([P, KI, F], bf16)
            xp_sb = bp.tile([P, KH, F], bf16)
            # h_i (i = t+1, 0..s) lives in parity tensor (i % 2) at col-slot
            # (i // 2) * BL; the A tensor holds hid-tile 0, B holds tiles 1..3.
            n0 = (s // 2 + 1) * BL
            n1 = (s // 2) * BL
            hs0A = bp.tile([P, 1, n0], bf16)
            hs0B = bp.tile([P, KH - 1, n0], bf16)
            hs1A = bp.tile([P, 1, n1], bf16)
            hs1B = bp.tile([P, KH - 1, n1], bf16)
            hA = [hs0A, hs1A]
            hB = [hs0B, hs1B]
            out_sb = bp.tile([P, KO, 2, F2], f32)

            # ---- constant loads (gpsimd = SWDGE does the fp32->bf16 casts) ----
            nc.gpsimd.dma_start(wih_sb[:], wih[:].rearrange("(k p) h -> p k h", p=P))
            nc.gpsimd.dma_start(whh_sb[:], whh[:].rearrange("(k p) h -> p k h", p=P))
            nc.gpsimd.dma_start(wff_sb[:], wff[:].rearrange("(k p) o -> p k o", p=P))
            nc.sync.dma_start(bcb_sb[:], bcb[:])
            nc.sync.dma_start(bfb_sb[:], bfb[:])
            nc.gpsimd.dma_start(eye_sb[:], eye[:])
            h0r = h0t[:].rearrange("(k p) b -> p k b", p=P)
            nc.gpsimd.dma_start(hs0A[:, :, 0:BL], h0r[:, 0:1, :])
            nc.gpsimd.dma_start(hs0B[:, :, 0:BL], h0r[:, 1:KH, :])

            xt_r = xt[:].rearrange("(k p) f -> p k f", p=P)

            with (
                tc.tile_pool(name="g1ps", bufs=2, space=bass.MemorySpace.PSUM) as g1p,
                tc.tile_pool(name="z0ps", bufs=2, space=bass.MemorySpace.PSUM) as z0p,
                tc.tile_pool(name="zBps", bufs=1, space=bass.MemorySpace.PSUM) as zBp,
            ):
                # ---- GEMM1: xp[hid, (t,b)] = W_ih @ x^T + (b_ih + b_hh) ----
                for j in range(nch):
                    sl = slice(j * ch, (j + 1) * ch)
                    nc.gpsimd.dma_start(xt_sb[:, :, sl], xt_r[:, :, sl])
                    for m in range(KH):
                        ps = g1p.tile([P, ch], f32)
                        for k in range(KI):
                            nc.tensor.matmul(
                                ps[:],
                                wih_sb[:, k, m * P : (m + 1) * P],
                                xt_sb[:, k, sl],
                                start=(k == 0),
                                stop=(k == KI - 1),
                            )
                        nc.scalar.activation(
                            xp_sb[:, m, sl], ps[:], AF.Identity,
                            bias=bcb_sb[:, m : m + 1],
                        )

                # ---- recurrence ----
                for t in range(s):
                    z0 = z0p.tile([P, 512], f32)
                    zB = zBp.tile([P, KH - 1, 512], f32)
                    rA, rB = hA[t % 2], hB[t % 2]
                    wA, wB = hA[(t + 1) % 2], hB[(t + 1) % 2]
                    rof = (t // 2) * BL
                    wof = ((t + 1) // 2) * BL
                    for m in range(KH):
                        zt = z0[:, 0:BL] if m == 0 else zB[:, m - 1, 0:BL]
                        nc.tensor.matmul(
                            zt,
                            eye_sb[:],
                            xp_sb[:, m, t * BL : (t + 1) * BL],
                            start=True,
                            stop=False,
                        )
                        for k in range(KH):
                            rhs = (
                                rA[:, 0, rof : rof + BL]
                                if k == 0
                                else rB[:, k - 1, rof : rof + BL]
                            )
                            nc.tensor.matmul(
                                zt,
                                whh_sb[:, k, m * P : (m + 1) * P],
                                rhs,
                                start=False,
                                stop=(k == KH - 1),
                            )
                        if m == 0:
                            nc.scalar.activation(
                                wA[:, 0, wof : wof + BL], z0[:, 0:BL], AF.Tanh
                            )
                    nc.scalar.activation(
                        wB[:, :, wof : wof + BL], zB[:, :, 0:BL], AF.Tanh
                    )

            # ---- GEMM2: y = W_ff @ hs + b_ff, per h-parity buffer ----
            # par=0 reads parity-0 slots 1.. (t = 2q+1); par=1 reads parity-1
            # slots 0.. (t = 2q)
            y_r = y[:].rearrange("o p q f -> p o q f")
            with tc.tile_pool(name="g2ps", bufs=3, space=bass.MemorySpace.PSUM) as g2p:
                for par in range(2):
                    pA, pB = hA[par], hB[par]
                    base = BL if par == 0 else 0
                    for j in range(nch2):
                        sl = slice(j * ch2, (j + 1) * ch2)
                        hsl = slice(base + j * ch2, base + (j + 1) * ch2)
                        for ot in range(KO):
                            ps = g2p.tile([P, ch2], f32)
                            for k in range(KH):
                                rhs = pA[:, 0, hsl] if k == 0 else pB[:, k - 1, hsl]
                                nc.tensor.matmul(
                                    ps[:],
                                    wff_sb[:, k, ot * P : (ot + 1) * P],
                                    rhs,
                                    start=(k == 0),
                                    stop=(k == KH - 1),
                                )
                            nc.scalar.activation(
                                out_sb[:, ot, par, sl], ps[:], AF.Identity,
                                bias=bfb_sb[:, ot : ot + 1],
                            )
                        nc.sync.dma_start(y_r[:, :, par, sl], out_sb[:, :, par, sl])

    return nc


def make_in_maps(x, h0, W_ih, W_hh, b_ih, b_hh, W_ff, b_ff, seq_len=S):
    """Host-side sharding + layout prep: per-core input dicts."""
    x = np.asarray(x, np.float32)
    h0 = np.asarray(h0, np.float32)
    wih = np.ascontiguousarray(np.asarray(W_ih, np.float32).T)  # [I, H]
    whh = np.ascontiguousarray(np.asarray(W_hh, np.float32).T)  # [H, H]
    wff = np.ascontiguousarray(np.asarray(W_ff, np.float32).T)  # [H, O]
    bc = np.asarray(b_ih, np.float32) + np.asarray(b_hh, np.float32)
    bcb = np.ascontiguousarray(bc.reshape(KH, P).T)             # [128, KH]
    bfb = np.ascontiguousarray(np.asarray(b_ff, np.float32).reshape(KO, P).T)
    eye = np.eye(P, dtype=np.float32)

    in_maps = []
    for c in range(NCORES):
        xs = x[c * BL : (c + 1) * BL, :seq_len]                 # [BL, s, I]
        xt = np.ascontiguousarray(xs.transpose(2, 1, 0)).reshape(I, seq_len * BL)
        h0t = np.ascontiguousarray(h0[c * BL : (c + 1) * BL].T)  # [H, BL]
        in_maps.append(
            {
                "xt": xt,
                "h0t": h0t,
                "wih": wih,
                "whh": whh,
                "wff": wff,
                "bcb": bcb,
                "bfb": bfb,
                "eye": eye,
            }
        )
    return in_maps


def assemble_output(results, seq_len=S):
    """Per-core y [KO, 128, 2, (s/2)*BL] -> full [B, s, O]."""
    s = seq_len
    outs = []
    for r in results:
        yc = np.asarray(r["y"]).reshape(O, 2, s // 2, BL)
        full = np.empty((O, s, BL), np.float32)
        full[:, 1::2, :] = yc[:, 0]   # par=0: t = 2q+1
        full[:, 0::2, :] = yc[:, 1]   # par=1: t = 2q
        outs.append(full.transpose(2, 1, 0))
    return np.ascontiguousarray(np.concatenate(outs, axis=0))


def _get_finalized_nc(seq_len=S):
    key = ("nc", seq_len)
    if key not in _builder_cache:
        nc = build_nc(seq_len)
        nc.finalize()
        _builder_cache[key] = nc
    return _builder_cache[key]


def run_on_cores(inputs, seq_len=S, **kwargs):
    from concourse.bass_utils import run_bass_kernel_spmd

    nc = _get_finalized_nc(seq_len)
    in_maps = make_in_maps(**inputs, seq_len=seq_len)
    res = run_bass_kernel_spmd(nc, in_maps, core_ids=list(range(NCORES)), **kwargs)
    return res


def kernel(**inputs) -> np.ndarray:
    res = run_on_cores(inputs)
    return assemble_output(res.results)


# revision 12
# speedup vs baseline: 1.8817x; 1.0275x over previous
# Trainium2 Bass kernel for nn_Decoder (RNN decoder):
#   xp = x @ W_ih^T + b_ih + b_hh            (GEMM1, bf16)
#   h_t = tanh(xp_t + h_{t-1} @ W_hh^T)      (512-step recurrence, bf16 matmul)
#   y  = hs @ W_ff^T + b_ff                  (GEMM2, bf16)
#
# Sharding: data-parallel over batch, 8 batch rows per core, weights replicated.
# Layouts are hidden-on-partitions so the sequential recurrence needs no
# transposes: h is stored [hid(4x128 part-tiles), batch(8)].
#
# Recurrence structure per step (critical path):
#   - output hid-tile m=0 accumulates in its own PSUM tensor z0 (1 bank),
#     tiles m=1..3 in zB (3 banks) — separate tensors so the dependency
#     tracker never serializes PE writes against the other half's tanh read.
#   - per bank: identity-matmul injects xp_t (start=True, dep-free), then 4
#     W_hh^T k-tile matmuls accumulate W@h.
#   - tanh split 1+3 on ACT: tile 0's tanh (actA) lands early and unblocks
#     the next step's k=0 matmuls while the other banks still accumulate.
#   - h history is split across 4 tensors by (step parity) x (A/B half) for
#     minimal tracker deps; GEMM2 consumes all four.

import numpy as np

B, S, I, H, O = 64, 512, 256, 512, 256
NCORES = 8
BL = B // NCORES  # 8 batch rows per core
P = 128
KI, KH, KO = I // P, H // P, O // P  # 2, 4, 2
CH = 512                             # free-dim chunk for the big GEMMs
_builder_cache = {}


def build_nc(seq_len=S):
    """Build the (single-core SPMD) Bass program for sequence length seq_len."""
    import concourse.bass as bass
    import concourse.mybir as mybir
    import concourse.tile as tile
    from concourse import bacc

    f32 = mybir.dt.float32
    bf16 = mybir.dt.bfloat16
    AF = mybir.ActivationFunctionType

    s = seq_len
    assert s % 2 == 0
    F = s * BL               # free length of (t, b) axis
    nch = max(1, F // CH)    # chunks for GEMM1
    ch = F // nch
    F2 = (F // 2)            # per-parity free length for GEMM2
    nch2 = max(1, F2 // CH)
    ch2 = F2 // nch2

    nc = bacc.Bacc("TRN2")

    xt = nc.dram_tensor("xt", [I, F], f32, kind="ExternalInput")       # x^T  (in, t*BL+b)
    h0t = nc.dram_tensor("h0t", [H, BL], f32, kind="ExternalInput")    # h0^T (hid, b)
    wih = nc.dram_tensor("wih", [I, H], f32, kind="ExternalInput")     # W_ih^T
    whh = nc.dram_tensor("whh", [H, H], f32, kind="ExternalInput")     # W_hh^T
    wff = nc.dram_tensor("wff", [H, O], f32, kind="ExternalInput")     # W_ff^T
    bcb = nc.dram_tensor("bcb", [P, KH], f32, kind="ExternalInput")    # b_ih+b_hh, [128, 4]
    bfb = nc.dram_tensor("bfb", [P, KO], f32, kind="ExternalInput")    # b_ff, [128, 2]
    eye = nc.dram_tensor("eye", [P, P], f32, kind="ExternalInput")
    # y[ot, p, par, q*BL + b]:  par=0 -> t = 2q+1, par=1 -> t = 2q
    y = nc.dram_tensor("y", [KO, P, 2, F2], f32, kind="ExternalOutput")

    with tile.TileContext(nc) as tc:
        with (
            tc.tile_pool(name="const", bufs=1) as cp,
            tc.tile_pool(name="big", bufs=1) as bp,
        ):
            wih_sb = cp.tile([P, KI, H], bf16)
            whh_sb = cp.tile([P, KH, H], bf16)
            wff_sb = cp.tile([P, KH, O], bf16)
            bcb_sb = cp.tile([P, KH], f32)
            bfb_sb = cp.tile([P, KO], f32)
            eye_sb = cp.tile([P, P], bf16)

            xt_sb = bp.tile([P, KI, F], bf16)
            xp_sb = bp.tile([P, KH, F], bf16)
            # h_i (i = t+1, 0..s) lives in parity tensor (i % 2) at col-slot
            # (i // 2) * BL; the A tensor holds hid-tile 0, B holds tiles 1..3.
            n0 = (s // 2 + 1) * BL
            n1 = (s // 2) * BL
            hs0A = bp.tile([P, 1, n0], bf16)
            hs0B = bp.tile([P, KH - 1, n0], bf16)
            hs1A = bp.tile([P, 1, n1], bf16)
            hs1B = bp.tile([P, KH - 1, n1], bf16)
            hA = [hs0A, hs1A]
            hB = [hs0B, hs1B]
            out_sb = bp.tile([P, KO, 2, F2], f32)

            # ---- constant loads (gpsimd = SWDGE does the fp32->bf16 casts) ----
            nc.gpsimd.dma_start(wih_sb[:], wih[:].rearrange("(k p) h -> p k h", p=P))
            nc.gpsimd.dma_start(whh_sb[:], whh[:].rearrange("(k p) h -> p k h", p=P))
            nc.gpsimd.dma_start(wff_sb[:], wff[:].rearrange("(k p) o -> p k o", p=P))
            nc.sync.dma_start(bcb_sb[:], bcb[:])
            nc.sync.dma_start(bfb_sb[:], bfb[:])
            nc.gpsimd.dma_start(eye_sb[:], eye[:])
            h0r = h0t[:].rearrange("(k p) b -> p k b", p=P)
            nc.gpsimd.dma_start(hs0A[:, :, 0:BL], h0r[:, 0:1, :])
            nc.gpsimd.dma_start(hs0B[:, :, 0:BL], h0r[:, 1:KH, :])

            xt_r = xt[:].rearrange("(k p) f -> p k f", p=P)

            # ---- GEMM1: xp[hid, (t,b)] = W_ih @ x^T + (b_ih + b_hh) ----
            # Drains alternate DVE/ACT so neither engine's drain tail idles
            # the PE long enough to re-throttle the HAM clock gate.
            with tc.tile_pool(name="g1ps", bufs=2, space=bass.MemorySpace.PSUM) as g1p:
                for j in range(nch):
                    sl = slice(j * ch, (j + 1) * ch)
                    nc.gpsimd.dma_start(xt_sb[:, :, sl], xt_r[:, :, sl])
                    for m in range(KH):
                        ps = g1p.tile([P, ch], f32)
                        for k in range(KI):
                            nc.tensor.matmul(
                                ps[:],
                                wih_sb[:, k, m * P : (m + 1) * P],
                                xt_sb[:, k, sl],
                                start=(k == 0),
                                stop=(k == KI - 1),
                            )
                        if m % 2 == 0:
                            nc.vector.tensor_scalar_add(
                                xp_sb[:, m, sl], ps[:], bcb_sb[:, m : m + 1]
                            )
                        else:
                            nc.scalar.activation(
                                xp_sb[:, m, sl], ps[:], AF.Identity,
                                bias=bcb_sb[:, m : m + 1],
                            )

            # ---- recurrence ----
            with (
                tc.tile_pool(name="z0ps", bufs=1, space=bass.MemorySpace.PSUM) as z0p,
                tc.tile_pool(name="zBps", bufs=2, space=bass.MemorySpace.PSUM) as zBp,
                tc.tile_pool(name="wmps", bufs=1, space=bass.MemorySpace.PSUM) as wmp,
            ):
                warm_ps = wmp.tile([P, 512], f32)
                for t in range(s):
                    z0 = z0p.tile([P, 512], f32)
                    zB = zBp.tile([P, KH - 1, 512], f32)
                    rA, rB = hA[t % 2], hB[t % 2]
                    wA, wB = hA[(t + 1) % 2], hB[(t + 1) % 2]
                    rof = (t // 2) * BL
                    wof = ((t + 1) // 2) * BL
                    # zB banks first (they gate the big tanh actB -> next
                    # step); k=0 last inside each bank since its producer
                    # (actA) lands earlier than actB anyway.
                    for m in (1, 2, 3, 0):
                        zt = z0[:, 0:BL] if m == 0 else zB[:, m - 1, 0:BL]
                        nc.tensor.matmul(
                            zt,
                            eye_sb[:],
                            xp_sb[:, m, t * BL : (t + 1) * BL],
                            start=True,
                            stop=False,
                        )
                        for k in (1, 2, 3, 0):
                            rhs = (
                                rA[:, 0, rof : rof + BL]
                                if k == 0
                                else rB[:, k - 1, rof : rof + BL]
                            )
                            nc.tensor.matmul(
                                zt,
                                whh_sb[:, k, m * P : (m + 1) * P],
                                rhs,
                                start=False,
                                stop=(k == 0),
                            )
                        if m == 0:
                            nc.scalar.activation(
                                wA[:, 0, wof : wof + BL], z0[:, 0:BL], AF.Tanh
                            )
                    nc.scalar.activation(
                        wB[:, :, wof : wof + BL], zB[:, :, 0:BL], AF.Tanh
                    )
                    # Filler matmuls: keep the PE active through the tanh
                    # round-trip so the HAM clock gate stays at 2.4 GHz.
                    for _ in range(3):
                        nc.tensor.matmul(
                            warm_ps[:, 0:P],
                            whh_sb[:, 0, 0:P],
                            xp_sb[:, 0, 0:P],
                            start=True,
                            stop=True,
                        )

            # ---- GEMM2: y = W_ff @ hs + b_ff, per h-parity buffer ----
            # par=0 reads parity-0 slots 1.. (t = 2q+1); par=1 reads parity-1
            # slots 0.. (t = 2q)
            y_r = y[:].rearrange("o p q f -> p o q f")
            with tc.tile_pool(name="g2ps", bufs=3, space=bass.MemorySpace.PSUM) as g2p:
                for par in range(2):
                    pA, pB = hA[par], hB[par]
                    base = BL if par == 0 else 0
                    for j in range(nch2):
                        sl = slice(j * ch2, (j + 1) * ch2)
                        hsl = slice(base + j * ch2, base + (j + 1) * ch2)
                        for ot in range(KO):
                            ps = g2p.tile([P, ch2], f32)
                            for k in range(KH):
                                rhs = pA[:, 0, hsl] if k == 0 else pB[:, k - 1, hsl]
                                nc.tensor.matmul(
                                    ps[:],
                                    wff_sb[:, k, ot * P : (ot + 1) * P],
                                    rhs,
                                    start=(k == 0),
                                    stop=(k == KH - 1),
                                )
                            nc.scalar.activation(
                                out_sb[:, ot, par, sl], ps[:], AF.Identity,
                                bias=bfb_sb[:, ot : ot + 1],
                            )
                        nc.sync.dma_start(y_r[:, :, par, sl], out_sb[:, :, par, sl])

    return nc


def make_in_maps(x, h0, W_ih, W_hh, b_ih, b_hh, W_ff, b_ff, seq_len=S):
    """Host-side sharding + layout prep: per-core input dicts."""
    x = np.asarray(x, np.float32)
    h0 = np.asarray(h0, np.float32)
    wih = np.ascontiguousarray(np.asarray(W_ih, np.float32).T)  # [I, H]
    whh = np.ascontiguousarray(np.asarray(W_hh, np.float32).T)  # [H, H]
    wff = np.ascontiguousarray(np.asarray(W_ff, np.float32).T)  # [H, O]
    bc = np.asarray(b_ih, np.float32) + np.asarray(b_hh, np.float32)
    bcb = np.ascontiguousarray(bc.reshape(KH, P).T)             # [128, KH]
    bfb = np.ascontiguousarray(np.asarray(b_ff, np.float32).reshape(KO, P).T)
    eye = np.eye(P, dtype=np.float32)

    in_maps = []
    for c in range(NCORES):
        xs = x[c * BL : (c + 1) * BL, :seq_len]                 # [BL, s, I]
        xt = np.ascontiguousarray(xs.transpose(2, 1, 0)).reshape(I, seq_len * BL)
        h0t = np.ascontiguousarray(h0[c * BL : (c + 1) * BL].T)  # [H, BL]
        in_maps.append(
            {
                "xt": xt,
                "h0t": h0t,
                "wih": wih,
                "whh": whh,
                "wff": wff,
                "bcb": bcb,
                "bfb": bfb,
                "eye": eye,
            }
        )
    return in_maps


def assemble_output(results, seq_len=S):
    """Per-core y [KO, 128, 2, (s/2)*BL] -> full [B, s, O]."""
    s = seq_len
    outs = []
    for r in results:
        yc = np.asarray(r["y"]).reshape(O, 2, s // 2, BL)
        full = np.empty((O, s, BL), np.float32)
        full[:, 1::2, :] = yc[:, 0]   # par=0: t = 2q+1
        full[:, 0::2, :] = yc[:, 1]   # par=1: t = 2q
        outs.append(full.transpose(2, 1, 0))
    return np.ascontiguousarray(np.concatenate(outs, axis=0))


def _get_finalized_nc(seq_len=S):
    key = ("nc", seq_len)
    if key not in _builder_cache:
        nc = build_nc(seq_len)
        nc.finalize()
        _builder_cache[key] = nc
    return _builder_cache[key]


def run_on_cores(inputs, seq_len=S, **kwargs):
    from concourse.bass_utils import run_bass_kernel_spmd

    nc = _get_finalized_nc(seq_len)
    in_maps = make_in_maps(**inputs, seq_len=seq_len)
    res = run_bass_kernel_spmd(nc, in_maps, core_ids=list(range(NCORES)), **kwargs)
    return res


def kernel(**inputs) -> np.ndarray:
    res = run_on_cores(inputs)
    return assemble_output(res.results)


# revision 13
# speedup vs baseline: 1.9738x; 1.0489x over previous
# Trainium2 Bass kernel for nn_Decoder (RNN decoder):
#   xp = x @ W_ih^T + b_ih + b_hh            (GEMM1, bf16)
#   h_t = tanh(xp_t + h_{t-1} @ W_hh^T)      (512-step recurrence, bf16 matmul)
#   y  = hs @ W_ff^T + b_ff                  (GEMM2, bf16)
#
# Sharding: data-parallel over batch, 8 batch rows per core, weights replicated.
# Layouts are hidden-on-partitions so the sequential recurrence needs no
# transposes: h is stored [hid(4x128 part-tiles), batch(8)].
#
# Recurrence structure per step (critical path):
#   - output hid-tile m=0 accumulates in its own PSUM tensor z0 (1 bank),
#     tiles m=1..3 in zB (3 banks) — separate tensors so the dependency
#     tracker never serializes PE writes against the other half's tanh read.
#   - per bank: identity-matmul injects xp_t (start=True, dep-free), then 4
#     W_hh^T k-tile matmuls accumulate W@h.
#   - tanh split 1+3 on ACT: tile 0's tanh (actA) lands early and unblocks
#     the next step's k=0 matmuls while the other banks still accumulate.
#   - h history is split across 4 tensors by (step parity) x (A/B half) for
#     minimal tracker deps; GEMM2 consumes all four.

import numpy as np

B, S, I, H, O = 64, 512, 256, 512, 256
NCORES = 8
BL = B // NCORES  # 8 batch rows per core
P = 128
KI, KH, KO = I // P, H // P, O // P  # 2, 4, 2
CH = 512                             # free-dim chunk for the big GEMMs
_builder_cache = {}


def build_nc(seq_len=S):
    """Build the (single-core SPMD) Bass program for sequence length seq_len."""
    import concourse.bass as bass
    import concourse.mybir as mybir
    import concourse.tile as tile
    from concourse import bacc

    f32 = mybir.dt.float32
    bf16 = mybir.dt.bfloat16
    AF = mybir.ActivationFunctionType

    s = seq_len
    assert s % 2 == 0
    F = s * BL               # free length of (t, b) axis
    nch = max(1, F // CH)    # chunks for GEMM1
    ch = F // nch
    F2 = (F // 2)            # per-parity free length for GEMM2
    nch2 = max(1, F2 // CH)
    ch2 = F2 // nch2

    nc = bacc.Bacc("TRN2")

    xt = nc.dram_tensor("xt", [I, F], f32, kind="ExternalInput")       # x^T  (in, t*BL+b)
    h0t = nc.dram_tensor("h0t", [H, BL], f32, kind="ExternalInput")    # h0^T (hid, b)
    wih = nc.dram_tensor("wih", [I, H], f32, kind="ExternalInput")     # W_ih^T
    whh = nc.dram_tensor("whh", [H, H], f32, kind="ExternalInput")     # W_hh^T
    wff = nc.dram_tensor("wff", [H, O], f32, kind="ExternalInput")     # W_ff^T
    bcb = nc.dram_tensor("bcb", [P, KH], f32, kind="ExternalInput")    # b_ih+b_hh, [128, 4]
    bfb = nc.dram_tensor("bfb", [P, KO], f32, kind="ExternalInput")    # b_ff, [128, 2]
    eye = nc.dram_tensor("eye", [P, P], f32, kind="ExternalInput")
    # y[ot, p, par, q*BL + b]:  par=0 -> t = 2q+1, par=1 -> t = 2q
    y = nc.dram_tensor("y", [KO, P, 2, F2], f32, kind="ExternalOutput")

    with tile.TileContext(nc) as tc:
        with (
            tc.tile_pool(name="const", bufs=1) as cp,
            tc.tile_pool(name="big", bufs=1) as bp,
        ):
            wih_sb = cp.tile([P, KI, H], bf16)
            whh_sb = cp.tile([P, KH, H], bf16)
            wff_sb = cp.tile([P, KH, O], bf16)
            bcb_sb = cp.tile([P, KH], f32)
            bfb_sb = cp.tile([P, KO], f32)
            eye_sb = cp.tile([P, P], bf16)

            xt_sb = bp.tile([P, KI, F], bf16)
            xp_sb = bp.tile([P, KH, F], bf16)
            # h_i (i = t+1, 0..s) lives in parity tensor (i % 2) at col-slot
            # (i // 2) * BL; the A tensor holds hid-tile 0, B holds tiles 1..3.
            n0 = (s // 2 + 1) * BL
            n1 = (s // 2) * BL
            hs0A = bp.tile([P, 1, n0], bf16)
            hs0B = bp.tile([P, KH - 1, n0], bf16)
            hs1A = bp.tile([P, 1, n1], bf16)
            hs1B = bp.tile([P, KH - 1, n1], bf16)
            hA = [hs0A, hs1A]
            hB = [hs0B, hs1B]
            out_sb = bp.tile([P, KO, 2, F2], f32)

            # ---- constant loads (gpsimd = SWDGE does the fp32->bf16 casts) ----
            nc.gpsimd.dma_start(wih_sb[:], wih[:].rearrange("(k p) h -> p k h", p=P))
            nc.gpsimd.dma_start(whh_sb[:], whh[:].rearrange("(k p) h -> p k h", p=P))
            nc.gpsimd.dma_start(wff_sb[:], wff[:].rearrange("(k p) o -> p k o", p=P))
            nc.sync.dma_start(bcb_sb[:], bcb[:])
            nc.sync.dma_start(bfb_sb[:], bfb[:])
            nc.gpsimd.dma_start(eye_sb[:], eye[:])
            h0r = h0t[:].rearrange("(k p) b -> p k b", p=P)
            nc.gpsimd.dma_start(hs0A[:, :, 0:BL], h0r[:, 0:1, :])
            nc.gpsimd.dma_start(hs0B[:, :, 0:BL], h0r[:, 1:KH, :])

            xt_r = xt[:].rearrange("(k p) f -> p k f", p=P)

            # ---- GEMM1: xp[hid, (t,b)] = W_ih @ x^T + (b_ih + b_hh) ----
            # Drains alternate DVE/ACT so neither engine's drain tail idles
            # the PE long enough to re-throttle the HAM clock gate.
            with tc.tile_pool(name="g1ps", bufs=2, space=bass.MemorySpace.PSUM) as g1p:
                for j in range(nch):
                    sl = slice(j * ch, (j + 1) * ch)
                    nc.gpsimd.dma_start(xt_sb[:, :, sl], xt_r[:, :, sl])
                    for m in range(KH):
                        ps = g1p.tile([P, ch], f32)
                        for k in range(KI):
                            nc.tensor.matmul(
                                ps[:],
                                wih_sb[:, k, m * P : (m + 1) * P],
                                xt_sb[:, k, sl],
                                start=(k == 0),
                                stop=(k == KI - 1),
                            )
                        if m % 2 == 0:
                            nc.vector.tensor_scalar_add(
                                xp_sb[:, m, sl], ps[:], bcb_sb[:, m : m + 1]
                            )
                        else:
                            nc.scalar.activation(
                                xp_sb[:, m, sl], ps[:], AF.Identity,
                                bias=bcb_sb[:, m : m + 1],
                            )

            # ---- recurrence ----
            with (
                tc.tile_pool(name="z0ps", bufs=1, space=bass.MemorySpace.PSUM) as z0p,
                tc.tile_pool(name="zBps", bufs=2, space=bass.MemorySpace.PSUM) as zBp,
            ):
                for t in range(s):
                    z0 = z0p.tile([P, 512], f32)
                    zB = zBp.tile([P, KH - 1, 512], f32)
                    rA, rB = hA[t % 2], hB[t % 2]
                    wA, wB = hA[(t + 1) % 2], hB[(t + 1) % 2]
                    rof = (t // 2) * BL
                    wof = ((t + 1) // 2) * BL

                    def kmm(m, k):
                        zt = z0[:, 0:BL] if m == 0 else zB[:, m - 1, 0:BL]
                        rhs = (
                            rA[:, 0, rof : rof + BL]
                            if k == 0
                            else rB[:, k - 1, rof : rof + BL]
                        )
                        nc.tensor.matmul(
                            zt,
                            whh_sb[:, k, m * P : (m + 1) * P],
                            rhs,
                            start=False,
                            stop=(k == 0),
                        )

                    def imm(m):
                        zt = z0[:, 0:BL] if m == 0 else zB[:, m - 1, 0:BL]
                        nc.tensor.matmul(
                            zt,
                            eye_sb[:],
                            xp_sb[:, m, t * BL : (t + 1) * BL],
                            start=True,
                            stop=False,
                        )

                    # zB banks (m=1..3) gate actB which gates the next step's
                    # k=1..3 matmuls — their chain is the critical cycle.
                    # I-matmuls are dep-free (hoisted by the scheduler); the
                    # k=0 matmuls come last since their producer (actA) runs
                    # second on the ACT engine.
                    for m in (1, 2, 3):
                        imm(m)
                    for k in (1, 2, 3):
                        for m in (1, 2, 3):
                            kmm(m, k)
                    for m in (1, 2, 3):
                        kmm(m, 0)
                    nc.scalar.activation(
                        wB[:, :, wof : wof + BL], zB[:, :, 0:BL], AF.Tanh
                    )
                    imm(0)
                    for k in (1, 2, 3, 0):
                        kmm(0, k)
                    nc.scalar.activation(
                        wA[:, 0, wof : wof + BL], z0[:, 0:BL], AF.Tanh
                    )

            # ---- GEMM2: y = W_ff @ hs + b_ff, per h-parity buffer ----
            # par=0 reads parity-0 slots 1.. (t = 2q+1); par=1 reads parity-1
            # slots 0.. (t = 2q)
            y_r = y[:].rearrange("o p q f -> p o q f")
            with tc.tile_pool(name="g2ps", bufs=3, space=bass.MemorySpace.PSUM) as g2p:
                for par in range(2):
                    pA, pB = hA[par], hB[par]
                    base = BL if par == 0 else 0
                    for j in range(nch2):
                        sl = slice(j * ch2, (j + 1) * ch2)
                        hsl = slice(base + j * ch2, base + (j + 1) * ch2)
                        for ot in range(KO):
                            ps = g2p.tile([P, ch2], f32)
                            for k in range(KH):
                                rhs = pA[:, 0, hsl] if k == 0 else pB[:, k - 1, hsl]
                                nc.tensor.matmul(
                                    ps[:],
                                    wff_sb[:, k, ot * P : (ot + 1) * P],
                                    rhs,
                                    start=(k == 0),
                                    stop=(k == KH - 1),
                                )
                            nc.scalar.activation(
                                out_sb[:, ot, par, sl], ps[:], AF.Identity,
                                bias=bfb_sb[:, ot : ot + 1],
                            )
                        nc.sync.dma_start(y_r[:, :, par, sl], out_sb[:, :, par, sl])

    return nc


def make_in_maps(x, h0, W_ih, W_hh, b_ih, b_hh, W_ff, b_ff, seq_len=S):
    """Host-side sharding + layout prep: per-core input dicts."""
    x = np.asarray(x, np.float32)
    h0 = np.asarray(h0, np.float32)
    wih = np.ascontiguousarray(np.asarray(W_ih, np.float32).T)  # [I, H]
    whh = np.ascontiguousarray(np.asarray(W_hh, np.float32).T)  # [H, H]
    wff = np.ascontiguousarray(np.asarray(W_ff, np.float32).T)  # [H, O]
    bc = np.asarray(b_ih, np.float32) + np.asarray(b_hh, np.float32)
    bcb = np.ascontiguousarray(bc.reshape(KH, P).T)             # [128, KH]
    bfb = np.ascontiguousarray(np.asarray(b_ff, np.float32).reshape(KO, P).T)
    eye = np.eye(P, dtype=np.float32)

    in_maps = []
    for c in range(NCORES):
        xs = x[c * BL : (c + 1) * BL, :seq_len]                 # [BL, s, I]
        xt = np.ascontiguousarray(xs.transpose(2, 1, 0)).reshape(I, seq_len * BL)
        h0t = np.ascontiguousarray(h0[c * BL : (c + 1) * BL].T)  # [H, BL]
        in_maps.append(
            {
                "xt": xt,
                "h0t": h0t,
                "wih": wih,
                "whh": whh,
                "wff": wff,
                "bcb": bcb,
                "bfb": bfb,
                "eye": eye,
            }
        )
    return in_maps


def assemble_output(results, seq_len=S):
    """Per-core y [KO, 128, 2, (s/2)*BL] -> full [B, s, O]."""
    s = seq_len
    outs = []
    for r in results:
        yc = np.asarray(r["y"]).reshape(O, 2, s // 2, BL)
        full = np.empty((O, s, BL), np.float32)
        full[:, 1::2, :] = yc[:, 0]   # par=0: t = 2q+1
        full[:, 0::2, :] = yc[:, 1]   # par=1: t = 2q
        outs.append(full.transpose(2, 1, 0))
    return np.ascontiguousarray(np.concatenate(outs, axis=0))


def _get_finalized_nc(seq_len=S):
    key = ("nc", seq_len)
    if key not in _builder_cache:
        nc = build_nc(seq_len)
        nc.finalize()
        _builder_cache[key] = nc
    return _builder_cache[key]


def run_on_cores(inputs, seq_len=S, **kwargs):
    from concourse.bass_utils import run_bass_kernel_spmd

    nc = _get_finalized_nc(seq_len)
    in_maps = make_in_maps(**inputs, seq_len=seq_len)
    res = run_bass_kernel_spmd(nc, in_maps, core_ids=list(range(NCORES)), **kwargs)
    return res


def kernel(**inputs) -> np.ndarray:
    res = run_on_cores(inputs)
    return assemble_output(res.results)


# revision 17
# speedup vs baseline: 1.9888x; 1.0076x over previous
# Trainium2 Bass kernel for nn_Decoder (RNN decoder):
#   xp = x @ W_ih^T + b_ih + b_hh            (GEMM1, bf16)
#   h_t = tanh(xp_t + h_{t-1} @ W_hh^T)      (512-step recurrence, bf16 matmul)
#   y  = hs @ W_ff^T + b_ff                  (GEMM2, bf16)
#
# Sharding: data-parallel over batch, 8 batch rows per core, weights replicated.
# Layouts are hidden-on-partitions so the sequential recurrence needs no
# transposes: h is stored [hid(4x128 part-tiles), batch(8)].
#
# Recurrence structure per step (critical path):
#   - output hid-tile m=0 accumulates in its own PSUM tensor z0 (1 bank),
#     tiles m=1..3 in zB (3 banks, double-buffered) — separate tensors so the
#     dependency tracker never serializes PE writes against the other half's
#     tanh read.
#   - per bank: identity-matmul injects xp_t (start=True, dep-free, hoisted
#     into PE idle), then 4 W_hh^T k-tile matmuls accumulate W@h.
#   - tanh split 3+1 on ACT: actB (tiles 1-3) is the critical producer and is
#     scheduled first; actA (tile 0) trails and feeds the next step's k=0
#     matmuls, which are ordered last in the burst.
#   - h history is split across 4 tensors by (step parity) x (A/B half) for
#     minimal tracker deps; GEMM2 consumes all four.
#   - most of GEMM2 is interleaved into the recurrence's PE idle (1 matmul
#     per step from t=384), with drains on the otherwise-idle DVE.

import numpy as np
import ml_dtypes

B, S, I, H, O = 64, 512, 256, 512, 256
NCORES = 8
BL = B // NCORES  # 8 batch rows per core
P = 128
KI, KH, KO = I // P, H // P, O // P  # 2, 4, 2
CH = 512                             # free-dim chunk for GEMM1
_builder_cache = {}


def build_nc(seq_len=S):
    """Build the (single-core SPMD) Bass program for sequence length seq_len."""
    import concourse.bass as bass
    import concourse.mybir as mybir
    import concourse.tile as tile
    from concourse import bacc

    f32 = mybir.dt.float32
    bf16 = mybir.dt.bfloat16
    AF = mybir.ActivationFunctionType

    s = seq_len
    assert s % 2 == 0
    F = s * BL               # free length of (t, b) axis
    nch = max(1, F // CH)    # chunks for GEMM1
    ch = F // nch
    F2 = F // 2              # per-parity free length for GEMM2
    CH2 = 256
    nch2 = max(1, F2 // CH2)
    ch2 = F2 // nch2

    nc = bacc.Bacc("TRN2")

    xt = nc.dram_tensor("xt", [I, F], bf16, kind="ExternalInput")      # x^T  (in, t*BL+b)
    h0t = nc.dram_tensor("h0t", [H, BL], bf16, kind="ExternalInput")   # h0^T (hid, b)
    wih = nc.dram_tensor("wih", [I, H], bf16, kind="ExternalInput")    # W_ih^T
    whh = nc.dram_tensor("whh", [H, H], bf16, kind="ExternalInput")    # W_hh^T
    wff = nc.dram_tensor("wff", [H, O], bf16, kind="ExternalInput")    # W_ff^T
    bcb = nc.dram_tensor("bcb", [P, KH], f32, kind="ExternalInput")    # b_ih+b_hh, [128, 4]
    bfb = nc.dram_tensor("bfb", [P, KO], f32, kind="ExternalInput")    # b_ff, [128, 2]
    eye = nc.dram_tensor("eye", [P, P], bf16, kind="ExternalInput")
    # y[ot, p, par, q*BL + b]:  par=0 -> t = 2q+1, par=1 -> t = 2q
    y = nc.dram_tensor("y", [KO, P, 2, F2], f32, kind="ExternalOutput")

    with tile.TileContext(nc) as tc:
        with (
            tc.tile_pool(name="const", bufs=1) as cp,
            tc.tile_pool(name="big", bufs=1) as bp,
        ):
            wih_sb = cp.tile([P, KI, H], bf16)
            whh_sb = cp.tile([P, KH, H], bf16)
            wff_sb = cp.tile([P, KH, O], bf16)
            bcb_sb = cp.tile([P, KH], f32)
            bfb_sb = cp.tile([P, KO], f32)
            eye_sb = cp.tile([P, P], bf16)

            xt_sb = bp.tile([P, KI, F], bf16)
            xp_sb = bp.tile([P, KH, F], bf16)
            # h_i (i = t+1, 0..s) lives in parity tensor (i % 2) at col-slot
            # (i // 2) * BL; the A tensor holds hid-tile 0, B holds tiles 1..3.
            n0 = (s // 2 + 1) * BL
            n1 = (s // 2) * BL
            hs0A = bp.tile([P, 1, n0], bf16)
            hs0B = bp.tile([P, KH - 1, n0], bf16)
            hs1A = bp.tile([P, 1, n1], bf16)
            hs1B = bp.tile([P, KH - 1, n1], bf16)
            hA = [hs0A, hs1A]
            hB = [hs0B, hs1B]
            out_sb = bp.tile([P, KO, 2, F2], f32)

            # ---- input loads (all bf16 host-side, plain HWDGE) ----
            xt_r = xt[:].rearrange("(k p) f -> p k f", p=P)
            nc.sync.dma_start(xt_sb[:, :, 0:ch], xt_r[:, :, 0:ch])
            nc.sync.dma_start(wih_sb[:], wih[:].rearrange("(k p) h -> p k h", p=P))
            nc.sync.dma_start(bcb_sb[:], bcb[:])
            nc.sync.dma_start(eye_sb[:], eye[:])
            h0r = h0t[:].rearrange("(k p) b -> p k b", p=P)
            nc.sync.dma_start(hs0A[:, :, 0:BL], h0r[:, 0:1, :])
            nc.sync.dma_start(hs0B[:, :, 0:BL], h0r[:, 1:KH, :])
            nc.sync.dma_start(whh_sb[:], whh[:].rearrange("(k p) h -> p k h", p=P))
            nc.sync.dma_start(wff_sb[:], wff[:].rearrange("(k p) o -> p k o", p=P))
            nc.sync.dma_start(bfb_sb[:], bfb[:])

            # ---- GEMM1: xp[hid, (t,b)] = W_ih @ x^T + (b_ih + b_hh) ----
            # Drains alternate DVE/ACT so neither engine's drain tail idles
            # the PE long enough to matter.
            with tc.tile_pool(name="g1ps", bufs=2, space=bass.MemorySpace.PSUM) as g1p:
                for j in range(nch):
                    sl = slice(j * ch, (j + 1) * ch)
                    if j + 1 < nch:
                        sl2 = slice((j + 1) * ch, (j + 2) * ch)
                        nc.sync.dma_start(xt_sb[:, :, sl2], xt_r[:, :, sl2])
                    for m in range(KH):
                        ps = g1p.tile([P, ch], f32)
                        for k in range(KI):
                            nc.tensor.matmul(
                                ps[:],
                                wih_sb[:, k, m * P : (m + 1) * P],
                                xt_sb[:, k, sl],
                                start=(k == 0),
                                stop=(k == KI - 1),
                            )
                        if m % 2 == 0:
                            nc.vector.tensor_scalar_add(
                                xp_sb[:, m, sl], ps[:], bcb_sb[:, m : m + 1]
                            )
                        else:
                            nc.scalar.activation(
                                xp_sb[:, m, sl], ps[:], AF.Identity,
                                bias=bcb_sb[:, m : m + 1],
                            )

            # ---- recurrence, with most of GEMM2 interleaved ----
            y_r = y[:].rearrange("o p q f -> p o q f")
            # GEMM2 work units: each is (par, j2, ot) = 4 matmuls + 1 DVE
            # drain (+ the chunk's output DMA on the last ot).
            g2_jobs = [
                (par, j2, ot)
                for j2 in range(nch2)
                for par in range(2)
                for ot in range(KO)
            ]

            def g2_emit(job, g2p):
                par, j2, ot = job
                pA, pB = hA[par], hB[par]
                base = BL if par == 0 else 0
                sl = slice(j2 * ch2, (j2 + 1) * ch2)
                hsl = slice(base + j2 * ch2, base + (j2 + 1) * ch2)
                ps = g2p.tile([P, ch2], f32, tag="g2ps")
                for k in range(KH):
                    rhs = pA[:, 0, hsl] if k == 0 else pB[:, k - 1, hsl]
                    nc.tensor.matmul(
                        ps[:],
                        wff_sb[:, k, ot * P : (ot + 1) * P],
                        rhs,
                        start=(k == 0),
                        stop=(k == KH - 1),
                    )
                nc.vector.tensor_scalar_add(
                    out_sb[:, ot, par, sl], ps[:], bfb_sb[:, ot : ot + 1]
                )
                if ot == KO - 1:
                    nc.sync.dma_start(y_r[:, :, par, sl], out_sb[:, :, par, sl])
            # job i is legal once all h-slots it reads exist: chunk j2 covers
            # t <= 64*j2 + 63, i.e. after step 64*j2 + 63.
            def g2_ready_step(job):
                par, j2, ot = job
                return 64 * (j2 + 1)

            with (
                tc.tile_pool(name="z0ps", bufs=1, space=bass.MemorySpace.PSUM) as z0p,
                tc.tile_pool(name="zBps", bufs=2, space=bass.MemorySpace.PSUM) as zBp,
                tc.tile_pool(name="g2ps", bufs=1, space=bass.MemorySpace.PSUM) as g2p,
            ):
                g2_i = 0
                for t in range(s):
                    z0 = z0p.tile([P, 512], f32)
                    zB = zBp.tile([P, KH - 1, 512], f32)
                    rA, rB = hA[t % 2], hB[t % 2]
                    wA, wB = hA[(t + 1) % 2], hB[(t + 1) % 2]
                    rof = (t // 2) * BL
                    wof = ((t + 1) // 2) * BL

                    def kmm(m, k):
                        zt = z0[:, 0:BL] if m == 0 else zB[:, m - 1, 0:BL]
                        rhs = (
                            rA[:, 0, rof : rof + BL]
                            if k == 0
                            else rB[:, k - 1, rof : rof + BL]
                        )
                        nc.tensor.matmul(
                            zt,
                            whh_sb[:, k, m * P : (m + 1) * P],
                            rhs,
                            start=False,
                            stop=(k == 0),
                        )

                    def imm(m):
                        zt = z0[:, 0:BL] if m == 0 else zB[:, m - 1, 0:BL]
                        nc.tensor.matmul(
                            zt,
                            eye_sb[:],
                            xp_sb[:, m, t * BL : (t + 1) * BL],
                            start=True,
                            stop=False,
                        )

                    for m in (1, 2, 3):
                        imm(m)
                    for k in (1, 2, 3):
                        for m in (1, 2, 3):
                            kmm(m, k)
                    for m in (1, 2, 3):
                        kmm(m, 0)
                    nc.scalar.activation(
                        wB[:, :, wof : wof + BL], zB[:, :, 0:BL], AF.Tanh
                    )
                    imm(0)
                    for k in (1, 2, 3, 0):
                        kmm(0, k)
                    nc.scalar.activation(
                        wA[:, 0, wof : wof + BL], z0[:, 0:BL], AF.Tanh
                    )
                    # one GEMM2 unit every few steps in the tanh shadow, once
                    # its input h-slots exist
                    if t >= 96 and t % 4 == 0 and g2_i < len(g2_jobs):
                        job = g2_jobs[g2_i]
                        if g2_ready_step(job) <= t:
                            g2_emit(job, g2p)
                            g2_i += 1
                # tail of GEMM2 (last chunks need the final steps)
                while g2_i < len(g2_jobs):
                    g2_emit(g2_jobs[g2_i], g2p)
                    g2_i += 1

    return nc


def make_in_maps(x, h0, W_ih, W_hh, b_ih, b_hh, W_ff, b_ff, seq_len=S):
    """Host-side sharding + layout prep: per-core input dicts."""
    bf = ml_dtypes.bfloat16
    x = np.asarray(x, np.float32)
    h0 = np.asarray(h0, np.float32)
    wih = np.ascontiguousarray(np.asarray(W_ih, np.float32).T).astype(bf)   # [I, H]
    whh = np.ascontiguousarray(np.asarray(W_hh, np.float32).T).astype(bf)   # [H, H]
    wff = np.ascontiguousarray(np.asarray(W_ff, np.float32).T).astype(bf)   # [H, O]
    bc = np.asarray(b_ih, np.float32) + np.asarray(b_hh, np.float32)
    bcb = np.ascontiguousarray(bc.reshape(KH, P).T)             # [128, KH]
    bfb = np.ascontiguousarray(np.asarray(b_ff, np.float32).reshape(KO, P).T)
    eye = np.eye(P, dtype=np.float32).astype(bf)

    in_maps = []
    for c in range(NCORES):
        xs = x[c * BL : (c + 1) * BL, :seq_len]                 # [BL, s, I]
        xt = np.ascontiguousarray(xs.transpose(2, 1, 0)).reshape(I, seq_len * BL)
        h0t = np.ascontiguousarray(h0[c * BL : (c + 1) * BL].T)  # [H, BL]
        in_maps.append(
            {
                "xt": xt.astype(bf),
                "h0t": h0t.astype(bf),
                "wih": wih,
                "whh": whh,
                "wff": wff,
                "bcb": bcb,
                "bfb": bfb,
                "eye": eye,
            }
        )
    return in_maps


def assemble_output(results, seq_len=S):
    """Per-core y [KO, 128, 2, (s/2)*BL] -> full [B, s, O]."""
    s = seq_len
    outs = []
    for r in results:
        yc = np.asarray(r["y"]).reshape(O, 2, s // 2, BL)
        full = np.empty((O, s, BL), np.float32)
        full[:, 1::2, :] = yc[:, 0]   # par=0: t = 2q+1
        full[:, 0::2, :] = yc[:, 1]   # par=1: t = 2q
        outs.append(full.transpose(2, 1, 0))
    return np.ascontiguousarray(np.concatenate(outs, axis=0))


def _get_finalized_nc(seq_len=S):
    key = ("nc", seq_len)
    if key not in _builder_cache:
        nc = build_nc(seq_len)
        nc.finalize()
        _builder_cache[key] = nc
    return _builder_cache[key]


def run_on_cores(inputs, seq_len=S, **kwargs):
    from concourse.bass_utils import run_bass_kernel_spmd

    nc = _get_finalized_nc(seq_len)
    in_maps = make_in_maps(**inputs, seq_len=seq_len)
    res = run_bass_kernel_spmd(nc, in_maps, core_ids=list(range(NCORES)), **kwargs)
    return res


def kernel(**inputs) -> np.ndarray:
    res = run_on_cores(inputs)
    return assemble_output(res.results)


# revision 21
# speedup vs baseline: 2.1961x; 1.1043x over previous
# Trainium2 Bass kernel for nn_Decoder (RNN decoder):
#   xp = x @ W_ih^T + b_ih + b_hh            (GEMM1, bf16)
#   h_t = tanh(xp_t + h_{t-1} @ W_hh^T)      (512-step recurrence, bf16 matmul)
#   y  = hs @ W_ff^T + b_ff                  (GEMM2, bf16)
#
# Sharding: data-parallel over batch, 8 batch rows per core, weights replicated.
# Layouts are hidden-on-partitions so the sequential recurrence needs no
# transposes: h is stored [hid(4x128 part-tiles), batch(8)].
#
# Recurrence structure per step (critical path):
#   - output hid-tile m=0 accumulates in its own PSUM tensor z0 (1 bank),
#     tiles m=1..3 in zB (3 banks, double-buffered) — separate tensors so the
#     dependency tracker never serializes PE writes against the other half's
#     tanh read.
#   - per bank: identity-matmul injects xp_t (start=True, dep-free, hoisted
#     into PE idle), then 4 W_hh^T k-tile matmuls accumulate W@h.
#   - tanh split 3+1 on ACT: actB (tiles 1-3) is the critical producer and is
#     scheduled first; actA (tile 0) trails and feeds the next step's k=0
#     matmuls, which are ordered last in the burst.
#   - h history is split across 4 tensors by (step parity) x (A/B half) for
#     minimal tracker deps; GEMM2 consumes all four.
#   - most of GEMM2 is interleaved into the recurrence's PE idle (1 matmul
#     per step from t=384), with drains on the otherwise-idle DVE.

import numpy as np
import ml_dtypes

B, S, I, H, O = 64, 512, 256, 512, 256
NCORES = 8
BL = B // NCORES  # 8 batch rows per core
P = 128
KI, KH, KO = I // P, H // P, O // P  # 2, 4, 2
CH = 512                             # free-dim chunk for GEMM1
_builder_cache = {}


def build_nc(seq_len=S):
    """Build the (single-core SPMD) Bass program for sequence length seq_len."""
    import concourse.bass as bass
    import concourse.mybir as mybir
    import concourse.tile as tile
    from concourse import bacc

    f32 = mybir.dt.float32
    bf16 = mybir.dt.bfloat16
    AF = mybir.ActivationFunctionType

    s = seq_len
    assert s % 2 == 0
    F = s * BL               # free length of (t, b) axis
    nch = max(1, F // CH)    # chunks for GEMM1
    ch = F // nch
    F2 = F // 2              # per-parity free length for GEMM2
    CH2 = 256
    nch2 = max(1, F2 // CH2)
    ch2 = F2 // nch2

    nc = bacc.Bacc("TRN2")

    xt = nc.dram_tensor("xt", [I, F], bf16, kind="ExternalInput")      # x^T  (in, t*BL+b)
    h0t = nc.dram_tensor("h0t", [H, BL], bf16, kind="ExternalInput")   # h0^T (hid, b)
    wih = nc.dram_tensor("wih", [I, H], bf16, kind="ExternalInput")    # W_ih^T
    whh = nc.dram_tensor("whh", [H, H], bf16, kind="ExternalInput")    # W_hh^T
    wff = nc.dram_tensor("wff", [H, O], bf16, kind="ExternalInput")    # W_ff^T
    bcb = nc.dram_tensor("bcb", [P, KH], f32, kind="ExternalInput")    # b_ih+b_hh, [128, 4]
    bfb = nc.dram_tensor("bfb", [P, KO], f32, kind="ExternalInput")    # b_ff, [128, 2]
    eye = nc.dram_tensor("eye", [P, P], bf16, kind="ExternalInput")
    # y[ot, p, par, q*BL + b]:  par=0 -> t = 2q+1, par=1 -> t = 2q
    y = nc.dram_tensor("y", [KO, P, 2, F2], f32, kind="ExternalOutput")

    with tile.TileContext(nc) as tc:
        with (
            tc.tile_pool(name="const", bufs=1) as cp,
            tc.tile_pool(name="big", bufs=1) as bp,
        ):
            wih_sb = cp.tile([P, KI, H], bf16)
            whh_sb = cp.tile([P, KH, H], bf16)
            wff_sb = cp.tile([P, KH, O], bf16)
            bcb_sb = cp.tile([P, KH], f32)
            bfb_sb = cp.tile([P, KO], f32)
            eye_sb = cp.tile([P, P], bf16)

            xt_sb = bp.tile([P, KI, F], bf16)
            xp_sb = bp.tile([P, KH, F], bf16)
            # h_i (i = t+1, 0..s) lives in parity tensor (i % 2) at col-slot
            # (i // 2) * BL; the A tensor holds hid-tile 0, B holds tiles 1..3.
            n0 = (s // 2 + 1) * BL
            n1 = (s // 2) * BL
            hs0A = bp.tile([P, 1, n0], bf16)
            hs0B = bp.tile([P, KH - 1, n0], bf16)
            hs1A = bp.tile([P, 1, n1], bf16)
            hs1B = bp.tile([P, KH - 1, n1], bf16)
            hA = [hs0A, hs1A]
            hB = [hs0B, hs1B]
            out_sb = bp.tile([P, KO, 2, F2], f32)

            # ---- input loads (all bf16 host-side, plain HWDGE) ----
            xt_r = xt[:].rearrange("(k p) f -> p k f", p=P)
            nc.sync.dma_start(xt_sb[:, :, 0:ch], xt_r[:, :, 0:ch])
            nc.sync.dma_start(wih_sb[:], wih[:].rearrange("(k p) h -> p k h", p=P))
            nc.sync.dma_start(bcb_sb[:], bcb[:])
            nc.sync.dma_start(eye_sb[:], eye[:])
            h0r = h0t[:].rearrange("(k p) b -> p k b", p=P)
            nc.sync.dma_start(hs0A[:, :, 0:BL], h0r[:, 0:1, :])
            nc.sync.dma_start(hs0B[:, :, 0:BL], h0r[:, 1:KH, :])
            nc.sync.dma_start(whh_sb[:], whh[:].rearrange("(k p) h -> p k h", p=P))
            nc.sync.dma_start(wff_sb[:], wff[:].rearrange("(k p) o -> p k o", p=P))
            nc.sync.dma_start(bfb_sb[:], bfb[:])

            # ---- GEMM1: xp[hid, (t,b)] = W_ih @ x^T + (b_ih + b_hh) ----
            # Drains alternate DVE/ACT so neither engine's drain tail idles
            # the PE long enough to matter.
            with tc.tile_pool(name="g1ps", bufs=4, space=bass.MemorySpace.PSUM) as g1p:
                for j in range(nch):
                    sl = slice(j * ch, (j + 1) * ch)
                    if j + 1 < nch:
                        sl2 = slice((j + 1) * ch, (j + 2) * ch)
                        nc.sync.dma_start(xt_sb[:, :, sl2], xt_r[:, :, sl2])
                    for m in range(KH):
                        ps = g1p.tile([P, ch], f32)
                        for k in range(KI):
                            nc.tensor.matmul(
                                ps[:],
                                wih_sb[:, k, m * P : (m + 1) * P],
                                xt_sb[:, k, sl],
                                start=(k == 0),
                                stop=(k == KI - 1),
                            )
                        if m % 2 == 0:
                            nc.vector.tensor_scalar_add(
                                xp_sb[:, m, sl], ps[:], bcb_sb[:, m : m + 1]
                            )
                        else:
                            nc.scalar.activation(
                                xp_sb[:, m, sl], ps[:], AF.Identity,
                                bias=bcb_sb[:, m : m + 1],
                            )

            # ---- recurrence, with most of GEMM2 interleaved ----
            y_r = y[:].rearrange("o p q f -> p o q f")
            # GEMM2 work units: each is (par, j2, ot) = 4 matmuls + 1 DVE
            # drain (+ the chunk's output DMA on the last ot).
            g2_jobs = [
                (par, j2, ot)
                for j2 in range(nch2)
                for par in range(2)
                for ot in range(KO)
            ]

            def g2_emit(job, g2p):
                par, j2, ot = job
                pA, pB = hA[par], hB[par]
                base = BL if par == 0 else 0
                sl = slice(j2 * ch2, (j2 + 1) * ch2)
                hsl = slice(base + j2 * ch2, base + (j2 + 1) * ch2)
                ps = g2p.tile([P, ch2], f32, tag="g2ps")
                for k in range(KH):
                    rhs = pA[:, 0, hsl] if k == 0 else pB[:, k - 1, hsl]
                    nc.tensor.matmul(
                        ps[:],
                        wff_sb[:, k, ot * P : (ot + 1) * P],
                        rhs,
                        start=(k == 0),
                        stop=(k == KH - 1),
                    )
                nc.vector.tensor_scalar_add(
                    out_sb[:, ot, par, sl], ps[:], bfb_sb[:, ot : ot + 1]
                )
                if ot == KO - 1:
                    nc.sync.dma_start(y_r[:, :, par, sl], out_sb[:, :, par, sl])
            # job i is legal once all h-slots it reads exist: chunk j2 covers
            # t <= 64*j2 + 63, i.e. after step 64*j2 + 63.
            def g2_ready_step(job):
                par, j2, ot = job
                return 64 * (j2 + 1)

            with (
                tc.tile_pool(name="z0ps", bufs=1, space=bass.MemorySpace.PSUM) as z0p,
                tc.tile_pool(name="zBps", bufs=2, space=bass.MemorySpace.PSUM) as zBp,
                tc.tile_pool(name="g2ps", bufs=1, space=bass.MemorySpace.PSUM) as g2p,
            ):
                from concourse.tile import add_dep_helper

                g2_i = 0
                prev_last_k0 = None
                for t in range(s):
                    z0 = z0p.tile([P, 512], f32)
                    zB = zBp.tile([P, KH - 1, 512], f32)
                    rA, rB = hA[t % 2], hB[t % 2]
                    wA, wB = hA[(t + 1) % 2], hB[(t + 1) % 2]
                    rof = (t // 2) * BL
                    wof = ((t + 1) // 2) * BL

                    def kmm(m, k):
                        zt = z0[:, 0:BL] if m == 0 else zB[:, m - 1, 0:BL]
                        rhs = (
                            rA[:, 0, rof : rof + BL]
                            if k == 0
                            else rB[:, k - 1, rof : rof + BL]
                        )
                        return nc.tensor.matmul(
                            zt,
                            whh_sb[:, k, m * P : (m + 1) * P],
                            rhs,
                            start=False,
                            stop=(k == 0),
                        )

                    def imm(m):
                        zt = z0[:, 0:BL] if m == 0 else zB[:, m - 1, 0:BL]
                        return nc.tensor.matmul(
                            zt,
                            eye_sb[:],
                            xp_sb[:, m, t * BL : (t + 1) * BL],
                            start=True,
                            stop=False,
                        )

                    for m in (1, 2, 3):
                        ei = imm(m)
                        if prev_last_k0 is not None:
                            # ordering-only: keep dep-free xp-inject matmuls
                            # from being scheduled ahead of the previous
                            # step's k=0 matmuls in the PE stream
                            add_dep_helper(ei.ins, prev_last_k0.ins, sync=False)
                    for k in (1, 2, 3):
                        for m in (1, 2, 3):
                            kmm(m, k)
                    for m in (1, 2, 3):
                        prev_last_k0 = kmm(m, 0)
                    nc.scalar.activation(
                        wB[:, :, wof : wof + BL], zB[:, :, 0:BL], AF.Tanh
                    )
                    imm(0)
                    for k in (1, 2, 3, 0):
                        kmm(0, k)
                    nc.scalar.activation(
                        wA[:, 0, wof : wof + BL], z0[:, 0:BL], AF.Tanh
                    )
                    # one GEMM2 unit every few steps in the tanh shadow, once
                    # its input h-slots exist
                    if t >= 96 and t % 4 == 0 and g2_i < len(g2_jobs):
                        job = g2_jobs[g2_i]
                        if g2_ready_step(job) <= t:
                            g2_emit(job, g2p)
                            g2_i += 1
                # tail of GEMM2 (last chunks need the final steps)
                while g2_i < len(g2_jobs):
                    g2_emit(g2_jobs[g2_i], g2p)
                    g2_i += 1

    return nc


def make_in_maps(x, h0, W_ih, W_hh, b_ih, b_hh, W_ff, b_ff, seq_len=S):
    """Host-side sharding + layout prep: per-core input dicts."""
    bf = ml_dtypes.bfloat16
    x = np.asarray(x, np.float32)
    h0 = np.asarray(h0, np.float32)
    wih = np.ascontiguousarray(np.asarray(W_ih, np.float32).T).astype(bf)   # [I, H]
    whh = np.ascontiguousarray(np.asarray(W_hh, np.float32).T).astype(bf)   # [H, H]
    wff = np.ascontiguousarray(np.asarray(W_ff, np.float32).T).astype(bf)   # [H, O]
    bc = np.asarray(b_ih, np.float32) + np.asarray(b_hh, np.float32)
    bcb = np.ascontiguousarray(bc.reshape(KH, P).T)             # [128, KH]
    bfb = np.ascontiguousarray(np.asarray(b_ff, np.float32).reshape(KO, P).T)
    eye = np.eye(P, dtype=np.float32).astype(bf)

    in_maps = []
    for c in range(NCORES):
        xs = x[c * BL : (c + 1) * BL, :seq_len]                 # [BL, s, I]
        xt = np.ascontiguousarray(xs.transpose(2, 1, 0)).reshape(I, seq_len * BL)
        h0t = np.ascontiguousarray(h0[c * BL : (c + 1) * BL].T)  # [H, BL]
        in_maps.append(
            {
                "xt": xt.astype(bf),
                "h0t": h0t.astype(bf),
                "wih": wih,
                "whh": whh,
                "wff": wff,
                "bcb": bcb,
                "bfb": bfb,
                "eye": eye,
            }
        )
    return in_maps


def assemble_output(results, seq_len=S):
    """Per-core y [KO, 128, 2, (s/2)*BL] -> full [B, s, O]."""
    s = seq_len
    outs = []
    for r in results:
        yc = np.asarray(r["y"]).reshape(O, 2, s // 2, BL)
        full = np.empty((O, s, BL), np.float32)
        full[:, 1::2, :] = yc[:, 0]   # par=0: t = 2q+1
        full[:, 0::2, :] = yc[:, 1]   # par=1: t = 2q
        outs.append(full.transpose(2, 1, 0))
    return np.ascontiguousarray(np.concatenate(outs, axis=0))


def _get_finalized_nc(seq_len=S):
    key = ("nc", seq_len)
    if key not in _builder_cache:
        nc = build_nc(seq_len)
        nc.finalize()
        _builder_cache[key] = nc
    return _builder_cache[key]


def run_on_cores(inputs, seq_len=S, **kwargs):
    from concourse.bass_utils import run_bass_kernel_spmd

    nc = _get_finalized_nc(seq_len)
    in_maps = make_in_maps(**inputs, seq_len=seq_len)
    res = run_bass_kernel_spmd(nc, in_maps, core_ids=list(range(NCORES)), **kwargs)
    return res


def kernel(**inputs) -> np.ndarray:
    res = run_on_cores(inputs)
    return assemble_output(res.results)


# revision 22
# speedup vs baseline: 2.2030x; 1.0031x over previous
# Trainium2 Bass kernel for nn_Decoder (RNN decoder):
#   xp = x @ W_ih^T + b_ih + b_hh            (GEMM1, bf16)
#   h_t = tanh(xp_t + h_{t-1} @ W_hh^T)      (512-step recurrence, bf16 matmul)
#   y  = hs @ W_ff^T + b_ff                  (GEMM2, bf16)
#
# Sharding: data-parallel over batch, 8 batch rows per core, weights replicated.
# Layouts are hidden-on-partitions so the sequential recurrence needs no
# transposes: h is stored [hid(4x128 part-tiles), batch(8)].
#
# Recurrence structure per step (critical path):
#   - output hid-tile m=0 accumulates in its own PSUM tensor z0 (1 bank),
#     tiles m=1..3 in zB (3 banks, double-buffered) — separate tensors so the
#     dependency tracker never serializes PE writes against the other half's
#     tanh read.
#   - per bank: identity-matmul injects xp_t (start=True, dep-free, hoisted
#     into PE idle), then 4 W_hh^T k-tile matmuls accumulate W@h.
#   - tanh split 3+1 on ACT: actB (tiles 1-3) is the critical producer and is
#     scheduled first; actA (tile 0) trails and feeds the next step's k=0
#     matmuls, which are ordered last in the burst.
#   - h history is split across 4 tensors by (step parity) x (A/B half) for
#     minimal tracker deps; GEMM2 consumes all four.
#   - most of GEMM2 is interleaved into the recurrence's PE idle (1 matmul
#     per step from t=384), with drains on the otherwise-idle DVE.

import numpy as np
import ml_dtypes

B, S, I, H, O = 64, 512, 256, 512, 256
NCORES = 8
BL = B // NCORES  # 8 batch rows per core
P = 128
KI, KH, KO = I // P, H // P, O // P  # 2, 4, 2
CH = 512                             # free-dim chunk for GEMM1
_builder_cache = {}


def build_nc(seq_len=S):
    """Build the (single-core SPMD) Bass program for sequence length seq_len."""
    import concourse.bass as bass
    import concourse.mybir as mybir
    import concourse.tile as tile
    from concourse import bacc

    f32 = mybir.dt.float32
    bf16 = mybir.dt.bfloat16
    AF = mybir.ActivationFunctionType

    s = seq_len
    assert s % 2 == 0
    F = s * BL               # free length of (t, b) axis
    nch = max(1, F // CH)    # chunks for GEMM1
    ch = F // nch
    F2 = F // 2              # per-parity free length for GEMM2
    CH2 = 256
    nch2 = max(1, F2 // CH2)
    ch2 = F2 // nch2

    nc = bacc.Bacc("TRN2")

    xt = nc.dram_tensor("xt", [I, F], bf16, kind="ExternalInput")      # x^T  (in, t*BL+b)
    h0t = nc.dram_tensor("h0t", [H, BL], bf16, kind="ExternalInput")   # h0^T (hid, b)
    wih = nc.dram_tensor("wih", [I, H], bf16, kind="ExternalInput")    # W_ih^T
    whh = nc.dram_tensor("whh", [H, H], bf16, kind="ExternalInput")    # W_hh^T
    wff = nc.dram_tensor("wff", [H, O], bf16, kind="ExternalInput")    # W_ff^T
    bcb = nc.dram_tensor("bcb", [P, KH], f32, kind="ExternalInput")    # b_ih+b_hh, [128, 4]
    bfb = nc.dram_tensor("bfb", [P, KO], f32, kind="ExternalInput")    # b_ff, [128, 2]
    eye = nc.dram_tensor("eye", [P, P], bf16, kind="ExternalInput")
    # y[ot, p, par, q*BL + b]:  par=0 -> t = 2q+1, par=1 -> t = 2q
    y = nc.dram_tensor("y", [KO, P, 2, F2], f32, kind="ExternalOutput")

    with tile.TileContext(nc) as tc:
        with (
            tc.tile_pool(name="const", bufs=1) as cp,
            tc.tile_pool(name="big", bufs=1) as bp,
        ):
            wih_sb = cp.tile([P, KI, H], bf16)
            whh_sb = cp.tile([P, KH, H], bf16)
            wff_sb = cp.tile([P, KH, O], bf16)
            bcb_sb = cp.tile([P, KH], f32)
            bfb_sb = cp.tile([P, KO], f32)
            eye_sb = cp.tile([P, P], bf16)

            xt_sb = bp.tile([P, KI, F], bf16)
            xp_sb = bp.tile([P, KH, F], bf16)
            # h_i (i = t+1, 0..s) lives in parity tensor (i % 2) at col-slot
            # (i // 2) * BL; the A tensor holds hid-tile 0, B holds tiles 1..3.
            n0 = (s // 2 + 1) * BL
            n1 = (s // 2) * BL
            hs0A = bp.tile([P, 1, n0], bf16)
            hs0B = bp.tile([P, KH - 1, n0], bf16)
            hs1A = bp.tile([P, 1, n1], bf16)
            hs1B = bp.tile([P, KH - 1, n1], bf16)
            hA = [hs0A, hs1A]
            hB = [hs0B, hs1B]
            out_sb = bp.tile([P, KO, 2, F2], f32)

            # ---- input loads (all bf16 host-side, plain HWDGE) ----
            xt_r = xt[:].rearrange("(k p) f -> p k f", p=P)
            nc.sync.dma_start(xt_sb[:, :, 0:ch], xt_r[:, :, 0:ch])
            nc.sync.dma_start(wih_sb[:], wih[:].rearrange("(k p) h -> p k h", p=P))
            nc.sync.dma_start(bcb_sb[:], bcb[:])
            nc.sync.dma_start(eye_sb[:], eye[:])
            h0r = h0t[:].rearrange("(k p) b -> p k b", p=P)
            nc.sync.dma_start(hs0A[:, :, 0:BL], h0r[:, 0:1, :])
            nc.sync.dma_start(hs0B[:, :, 0:BL], h0r[:, 1:KH, :])
            nc.sync.dma_start(whh_sb[:], whh[:].rearrange("(k p) h -> p k h", p=P))
            nc.sync.dma_start(wff_sb[:], wff[:].rearrange("(k p) o -> p k o", p=P))
            nc.sync.dma_start(bfb_sb[:], bfb[:])

            # ---- GEMM1: xp[hid, (t,b)] = W_ih @ x^T + (b_ih + b_hh) ----
            # Drains alternate DVE/ACT so neither engine's drain tail idles
            # the PE long enough to matter.
            with tc.tile_pool(name="g1ps", bufs=4, space=bass.MemorySpace.PSUM) as g1p:
                # PE warmup during the input-DMA wait: N=512 matmuls run at
                # half rate until the HAM clock gate sees ~3.4us of sustained
                # PE activity, so spend the DMA-bound window warming up.
                wm = g1p.tile([P, 512], f32, tag="ps")
                for _ in range(40):
                    nc.tensor.matmul(
                        wm[:, 0:P], eye_sb[:], eye_sb[:], start=True, stop=True
                    )
                for j in range(nch):
                    sl = slice(j * ch, (j + 1) * ch)
                    if j + 1 < nch:
                        sl2 = slice((j + 1) * ch, (j + 2) * ch)
                        nc.sync.dma_start(xt_sb[:, :, sl2], xt_r[:, :, sl2])
                    for m in range(KH):
                        ps = g1p.tile([P, ch], f32)
                        for k in range(KI):
                            nc.tensor.matmul(
                                ps[:],
                                wih_sb[:, k, m * P : (m + 1) * P],
                                xt_sb[:, k, sl],
                                start=(k == 0),
                                stop=(k == KI - 1),
                            )
                        if m % 2 == 0:
                            nc.vector.tensor_scalar_add(
                                xp_sb[:, m, sl], ps[:], bcb_sb[:, m : m + 1]
                            )
                        else:
                            nc.scalar.activation(
                                xp_sb[:, m, sl], ps[:], AF.Identity,
                                bias=bcb_sb[:, m : m + 1],
                            )

            # ---- recurrence, with most of GEMM2 interleaved ----
            y_r = y[:].rearrange("o p q f -> p o q f")
            # GEMM2 work units: each is (par, j2, ot) = 4 matmuls + 1 DVE
            # drain (+ the chunk's output DMA on the last ot).
            g2_jobs = [
                (par, j2, ot)
                for j2 in range(nch2)
                for par in range(2)
                for ot in range(KO)
            ]

            def g2_emit(job, g2p):
                par, j2, ot = job
                pA, pB = hA[par], hB[par]
                base = BL if par == 0 else 0
                sl = slice(j2 * ch2, (j2 + 1) * ch2)
                hsl = slice(base + j2 * ch2, base + (j2 + 1) * ch2)
                ps = g2p.tile([P, ch2], f32, tag="g2ps")
                for k in range(KH):
                    rhs = pA[:, 0, hsl] if k == 0 else pB[:, k - 1, hsl]
                    nc.tensor.matmul(
                        ps[:],
                        wff_sb[:, k, ot * P : (ot + 1) * P],
                        rhs,
                        start=(k == 0),
                        stop=(k == KH - 1),
                    )
                nc.vector.tensor_scalar_add(
                    out_sb[:, ot, par, sl], ps[:], bfb_sb[:, ot : ot + 1]
                )
                if ot == KO - 1:
                    nc.sync.dma_start(y_r[:, :, par, sl], out_sb[:, :, par, sl])
            # job i is legal once all h-slots it reads exist: chunk j2 covers
            # t <= 64*j2 + 63, i.e. after step 64*j2 + 63.
            def g2_ready_step(job):
                par, j2, ot = job
                return 64 * (j2 + 1)

            with (
                tc.tile_pool(name="z0ps", bufs=1, space=bass.MemorySpace.PSUM) as z0p,
                tc.tile_pool(name="zBps", bufs=2, space=bass.MemorySpace.PSUM) as zBp,
                tc.tile_pool(name="g2ps", bufs=1, space=bass.MemorySpace.PSUM) as g2p,
            ):
                from concourse.tile import add_dep_helper

                g2_i = 0
                prev_last_k0 = None
                for t in range(s):
                    z0 = z0p.tile([P, 512], f32)
                    zB = zBp.tile([P, KH - 1, 512], f32)
                    rA, rB = hA[t % 2], hB[t % 2]
                    wA, wB = hA[(t + 1) % 2], hB[(t + 1) % 2]
                    rof = (t // 2) * BL
                    wof = ((t + 1) // 2) * BL

                    def kmm(m, k):
                        zt = z0[:, 0:BL] if m == 0 else zB[:, m - 1, 0:BL]
                        rhs = (
                            rA[:, 0, rof : rof + BL]
                            if k == 0
                            else rB[:, k - 1, rof : rof + BL]
                        )
                        return nc.tensor.matmul(
                            zt,
                            whh_sb[:, k, m * P : (m + 1) * P],
                            rhs,
                            start=False,
                            stop=(k == 0),
                        )

                    def imm(m):
                        zt = z0[:, 0:BL] if m == 0 else zB[:, m - 1, 0:BL]
                        return nc.tensor.matmul(
                            zt,
                            eye_sb[:],
                            xp_sb[:, m, t * BL : (t + 1) * BL],
                            start=True,
                            stop=False,
                        )

                    for m in (1, 2, 3):
                        ei = imm(m)
                        if prev_last_k0 is not None:
                            # ordering-only: keep dep-free xp-inject matmuls
                            # from being scheduled ahead of the previous
                            # step's k=0 matmuls in the PE stream
                            add_dep_helper(ei.ins, prev_last_k0.ins, sync=False)
                    for k in (1, 2, 3):
                        for m in (1, 2, 3):
                            kmm(m, k)
                    for m in (1, 2, 3):
                        prev_last_k0 = kmm(m, 0)
                    nc.scalar.activation(
                        wB[:, :, wof : wof + BL], zB[:, :, 0:BL], AF.Tanh
                    )
                    imm(0)
                    for k in (1, 2, 3, 0):
                        kmm(0, k)
                    nc.scalar.activation(
                        wA[:, 0, wof : wof + BL], z0[:, 0:BL], AF.Tanh
                    )
                    # one GEMM2 unit every few steps in the tanh shadow, once
                    # its input h-slots exist
                    if t >= 96 and t % 4 == 0 and g2_i < len(g2_jobs):
                        job = g2_jobs[g2_i]
                        if g2_ready_step(job) <= t:
                            g2_emit(job, g2p)
                            g2_i += 1
                # tail of GEMM2 (last chunks need the final steps)
                while g2_i < len(g2_jobs):
                    g2_emit(g2_jobs[g2_i], g2p)
                    g2_i += 1

    return nc


def make_in_maps(x, h0, W_ih, W_hh, b_ih, b_hh, W_ff, b_ff, seq_len=S):
    """Host-side sharding + layout prep: per-core input dicts."""
    bf = ml_dtypes.bfloat16
    x = np.asarray(x, np.float32)
    h0 = np.asarray(h0, np.float32)
    wih = np.ascontiguousarray(np.asarray(W_ih, np.float32).T).astype(bf)   # [I, H]
    whh = np.ascontiguousarray(np.asarray(W_hh, np.float32).T).astype(bf)   # [H, H]
    wff = np.ascontiguousarray(np.asarray(W_ff, np.float32).T).astype(bf)   # [H, O]
    bc = np.asarray(b_ih, np.float32) + np.asarray(b_hh, np.float32)
    bcb = np.ascontiguousarray(bc.reshape(KH, P).T)             # [128, KH]
    bfb = np.ascontiguousarray(np.asarray(b_ff, np.float32).reshape(KO, P).T)
    eye = np.eye(P, dtype=np.float32).astype(bf)

    in_maps = []
    for c in range(NCORES):
        xs = x[c * BL : (c + 1) * BL, :seq_len]                 # [BL, s, I]
        xt = np.ascontiguousarray(xs.transpose(2, 1, 0)).reshape(I, seq_len * BL)
        h0t = np.ascontiguousarray(h0[c * BL : (c + 1) * BL].T)  # [H, BL]
        in_maps.append(
            {
                "xt": xt.astype(bf),
                "h0t": h0t.astype(bf),
                "wih": wih,
                "whh": whh,
                "wff": wff,
                "bcb": bcb,
                "bfb": bfb,
                "eye": eye,
            }
        )
    return in_maps


def assemble_output(results, seq_len=S):
    """Per-core y [KO, 128, 2, (s/2)*BL] -> full [B, s, O]."""
    s = seq_len
    outs = []
    for r in results:
        yc = np.asarray(r["y"]).reshape(O, 2, s // 2, BL)
        full = np.empty((O, s, BL), np.float32)
        full[:, 1::2, :] = yc[:, 0]   # par=0: t = 2q+1
        full[:, 0::2, :] = yc[:, 1]   # par=1: t = 2q
        outs.append(full.transpose(2, 1, 0))
    return np.ascontiguousarray(np.concatenate(outs, axis=0))


def _get_finalized_nc(seq_len=S):
    key = ("nc", seq_len)
    if key not in _builder_cache:
        nc = build_nc(seq_len)
        nc.finalize()
        _builder_cache[key] = nc
    return _builder_cache[key]


def run_on_cores(inputs, seq_len=S, **kwargs):
    from concourse.bass_utils import run_bass_kernel_spmd

    nc = _get_finalized_nc(seq_len)
    in_maps = make_in_maps(**inputs, seq_len=seq_len)
    res = run_bass_kernel_spmd(nc, in_maps, core_ids=list(range(NCORES)), **kwargs)
    return res


def kernel(**inputs) -> np.ndarray:
    res = run_on_cores(inputs)
    return assemble_output(res.results)
